# revision 1
# baseline (speedup 1.0000x reference)
"""Multi-head attention (bs=4, seq=2048, hidden=1024, 16 heads) on 8 trn2 cores.

Sharding: core = (batch b, head-group g) with 4 batches x 2 groups of 8 heads.
Each core computes QKV projections for its head slice, causal+padded softmax
attention, and a partial output projection; the host sums the two partial
outputs per batch and adds o_b.

Device layout (per core):
  xT   [1024, 2048]  hidden[b]^T           (host-transposed)
  wqT/wkT/wvT [1024, 512]  W[rows r]^T     (host-transposed slices)
  woT  [512, 1024]   o_w[:, r]^T
  qT/kT = W^T-projections in [o, s] layout; v in [s, o] layout with a ones
  column per head (augmented-V) so softmax denominators accumulate in the
  same PSUM bank as the attention output.
Scores are computed transposed [sk, sq] so the softmax sum is a matmul
reduction; exp runs on ScalarE with the padding mask as a per-partition bias;
the causal triangle is zeroed multiplicatively on VectorE after exp.
"""
import os
import sys

for _p in ("/opt/trn_rl_repo",):
    if _p not in sys.path:
        sys.path.insert(0, _p)

import numpy as np

HID = 1024
HEADS = 16
D = 64
BS = 4
SEQ = 2048
NCORES = 8
HG = 2            # head groups (tensor-parallel axis)
HPG = HEADS // HG  # 8 heads per core
OG = HPG * D       # 512 projection dims per core
KC = HID // 128    # 8 hidden chunks
TQ = 4             # sq tiles
TW = SEQ // TQ     # 512 queries per tile
SC = SEQ // 128    # 16 s chunks
SCALE = 1.0 / np.sqrt(D)

_compiled = None


def _chunks_for_tile(t):
    """(sk_chunk, col_offset, width) list for sq-tile t (causal structure)."""
    out = [(c, 0, TW) for c in range(4 * t)]
    for i in range(4):
        out.append((4 * t + i, 128 * i, TW - 128 * i))
    return out


def _build():
    import concourse.tile as tile
    from concourse import bacc, mybir

    F32 = mybir.dt.float32
    F32R = mybir.dt.float32r
    BF16 = mybir.dt.bfloat16
    AF = mybir.ActivationFunctionType
    Alu = mybir.AluOpType

    nc = bacc.Bacc("TRN2", target_bir_lowering=False, debug=False,
                   num_devices=NCORES)

    xT_d = nc.dram_tensor("xT", [HID, SEQ], F32R, kind="ExternalInput").ap()
    wqT_d = nc.dram_tensor("wqT", [HID, OG], F32R, kind="ExternalInput").ap()
    wkT_d = nc.dram_tensor("wkT", [HID, OG], F32R, kind="ExternalInput").ap()
    wvT_d = nc.dram_tensor("wvT", [HID, OG], F32R, kind="ExternalInput").ap()
    woT_d = nc.dram_tensor("woT", [OG, HID], F32R, kind="ExternalInput").ap()
    qb_d = nc.dram_tensor("qb", [128, 4], F32, kind="ExternalInput").ap()
    kb_d = nc.dram_tensor("kb", [128, 4], F32, kind="ExternalInput").ap()
    vb_d = nc.dram_tensor("vb", [1, OG], F32R, kind="ExternalInput").ap()
    kmask_d = nc.dram_tensor("kmask", [128, SC], F32, kind="ExternalInput").ap()
    out_d = nc.dram_tensor("out", [SEQ, HID], F32, kind="ExternalOutput").ap()

    with tile.TileContext(nc) as tc:
        with tc.tile_pool(name="const", bufs=1) as cp, \
             tc.tile_pool(name="qT", bufs=1) as qTp, \
             tc.tile_pool(name="kT", bufs=1) as kTp, \
             tc.tile_pool(name="v", bufs=1) as vp, \
             tc.tile_pool(name="attnT", bufs=1) as aTp:

            ones_f = cp.tile([128, 128], F32, tag="ones_f")
            nc.gpsimd.memset(ones_f[:, :], 1.0)
            ones = cp.tile([128, 128], F32R, tag="ones")
            nc.scalar.copy(ones[:, :], ones_f[:, :])
            # tri01[p, j] = 1 if j >= p else 0  (keep keys <= query)
            tri01_f = cp.tile([128, 128], F32, tag="tri01_f")
            nc.gpsimd.affine_select(tri01_f[:, :], ones_f[:, :],
                                    pattern=[[1, 128]],
                                    compare_op=Alu.is_ge, fill=0.0,
                                    base=0, channel_multiplier=-1)
            tri01 = cp.tile([128, 128], BF16, tag="tri01")
            nc.scalar.copy(tri01[:, :], tri01_f[:, :])
            qb_s = cp.tile([128, 4], F32, tag="qb")
            nc.sync.dma_start(qb_s[:, :], qb_d[:, :])
            kb_s = cp.tile([128, 4], F32, tag="kb")
            nc.sync.dma_start(kb_s[:, :], kb_d[:, :])
            vb_s = cp.tile([1, OG], F32R, tag="vb")
            nc.sync.dma_start(vb_s[:, :], vb_d[:, :])
            kmask_s = cp.tile([128, SC], F32, tag="km")
            nc.sync.dma_start(kmask_s[:, :], kmask_d[:, :])

            qT_t = [qTp.tile([128, SEQ], F32R, tag=f"qT{i}", name=f"qT{i}") for i in range(4)]
            kT_t = [kTp.tile([128, SEQ], F32R, tag=f"kT{i}", name=f"kT{i}") for i in range(4)]
            v_t = [vp.tile([128, HPG * 65], BF16, tag=f"v{i}", name=f"v{i}") for i in range(SC)]

            # ---------------- phase 1: projections (2 seq halves) ---------
            HSEQ = SEQ // 2
            for half in range(2):
                hs = half * HSEQ
                with tc.tile_pool(name=f"xT{half}", bufs=1) as xp:
                    xT_t = []
                    for kc in range(KC):
                        xt = xp.tile([128, HSEQ], F32R, tag=f"xT{kc}",
                                     name=f"xTh{half}_{kc}")
                        nc.sync.dma_start(
                            xt[:, :], xT_d[kc * 128:(kc + 1) * 128,
                                           hs:hs + HSEQ])
                        xT_t.append(xt)

                    with tc.tile_pool(name=f"wqk{half}", bufs=1) as wp, \
                         tc.tile_pool(name=f"ps1_{half}", bufs=6,
                                      space="PSUM") as ps1:
                        wq_t, wk_t = [], []
                        for kc in range(KC):
                            wq = wp.tile([128, OG], F32R, tag=f"wq{kc}",
                                         name=f"wqh{half}_{kc}")
                            nc.sync.dma_start(
                                wq[:, :], wqT_d[kc * 128:(kc + 1) * 128, :])
                            wq_t.append(wq)
                            wk = wp.tile([128, OG], F32R, tag=f"wk{kc}",
                                         name=f"wkh{half}_{kc}")
                            nc.sync.dma_start(
                                wk[:, :], wkT_d[kc * 128:(kc + 1) * 128, :])
                            wk_t.append(wk)

                        for w_t, o_t, bias in ((wq_t, qT_t, qb_s),
                                               (wk_t, kT_t, kb_s)):
                            for oc in range(4):
                                pts = [ps1.tile([128, TW], F32, tag="p1",
                                                name="p1")
                                       for _ in range(HSEQ // TW)]
                                for kc in range(KC):
                                    for t in range(HSEQ // TW):
                                        nc.tensor.matmul(
                                            pts[t][:, :],
                                            w_t[kc][:, oc * 128:(oc + 1) * 128],
                                            xT_t[kc][:, t * TW:(t + 1) * TW],
                                            start=(kc == 0),
                                            stop=(kc == KC - 1))
                                for t in range(HSEQ // TW):
                                    nc.scalar.activation(
                                        o_t[oc][:, hs + t * TW:hs + (t + 1) * TW],
                                        pts[t][:, :], AF.Identity,
                                        bias=bias[:, oc:oc + 1], scale=1.0)

                    with tc.tile_pool(name=f"wv{half}", bufs=1) as wvp, \
                         tc.tile_pool(name=f"ps1b{half}", bufs=6,
                                      space="PSUM") as ps1b:
                        wv_t = []
                        for kc in range(KC):
                            wv = wvp.tile([128, OG], F32R, tag=f"wv{kc}",
                                          name=f"wvh{half}_{kc}")
                            nc.sync.dma_start(
                                wv[:, :], wvT_d[kc * 128:(kc + 1) * 128, :])
                            wv_t.append(wv)

                        for sc in range(SC // 2):
                            scg = half * (SC // 2) + sc
                            pv = ps1b.tile([128, OG], F32, tag="pv", name="pv")
                            for kc in range(KC):
                                nc.tensor.matmul(
                                    pv[:, :],
                                    xT_t[kc][:, sc * 128:(sc + 1) * 128],
                                    wv_t[kc][:, :],
                                    start=(kc == 0), stop=False)
                            # + v_b via ones-outer-product
                            nc.tensor.matmul(pv[:, :], ones[0:1, :],
                                             vb_s[0:1, :],
                                             start=False, stop=True)
                            src = pv.rearrange("p (h c) -> p h c", c=64)
                            dst = v_t[scg].rearrange("p (h c) -> p h c", c=65)
                            nc.scalar.activation(dst[:, :, 0:64], src[:, :, :],
                                                 AF.Copy)
                            nc.scalar.activation(
                                dst[:, :, 64:65],
                                ones_f[:, 0:HPG].unsqueeze(2),
                                AF.Copy)

            # ---------------- phase 2: attention ----------------
            # sq windows of 1024; scores land in 2-bank PSUM tiles so exp
            # runs as one wide ACT op per sk-chunk.
            attnT_t = [aTp.tile([128, SEQ], F32R, tag=f"aT{i}", name=f"aT{i}") for i in range(4)]
            W = 1024
            with tc.tile_pool(name="ph2", bufs=1) as p2, \
                 tc.tile_pool(name="ps2", bufs=1, space="PSUM") as ps2:
                for tw in range(SEQ // W):
                    chunks = [(c, 0, W) for c in range(8 * tw)]
                    chunks += [(8 * tw + i, 128 * i, W - 128 * i)
                               for i in range(8)]
                    # last chunk writing each 512-half of the window
                    last0 = max(i for i, (_, off, _) in enumerate(chunks)
                                if off < 512)
                    last1 = len(chunks) - 1
                    for h in range(HPG):
                        hb = (h % 2) * 64
                        hc = h // 2
                        kslice = kT_t[hc]
                        at0 = ps2.tile([128, 512], F32, tag="at", bufs=3,
                                       name="at0")
                        at1 = ps2.tile([128, 512], F32, tag="at", bufs=3,
                                       name="at1")
                        ats = (at0, at1)
                        for idx, (c, off, w) in enumerate(chunks):
                            sp = ps2.tile([128, W], F32, tag="sc", bufs=2)
                            for lo, hi in ((off, 512), (max(off, 512), W)):
                                if lo >= hi:
                                    continue
                                nc.tensor.matmul(
                                    sp[:, lo:hi],
                                    kslice[hb:hb + 64, c * 128:(c + 1) * 128],
                                    qT_t[hc][hb:hb + 64, tw * W + lo:tw * W + hi],
                                    start=True, stop=True)
                            et = p2.tile([128, W], BF16, tag="E", bufs=4)
                            nc.scalar.activation(et[:, :w], sp[:, off:off + w],
                                                 AF.Exp,
                                                 bias=kmask_s[:, c:c + 1],
                                                 scale=SCALE)
                            if off or c == 8 * tw:  # diagonal chunk
                                nc.vector.tensor_mul(et[:, 0:128], et[:, 0:128],
                                                     tri01[:, :])
                            for half in range(2):
                                lo = max(off, half * 512)
                                hi = (half + 1) * 512
                                if lo >= hi:
                                    continue
                                nc.tensor.matmul(
                                    ats[half][0:65, lo - half * 512:512],
                                    v_t[c][:, h * 65:(h + 1) * 65],
                                    et[:, lo - off:hi - off],
                                    start=(idx == 0),
                                    stop=(idx == (last0, last1)[half]))
                        for half in range(2):
                            at = ats[half]
                            recip = p2.tile([128, 512], F32R, tag="rc", bufs=2)
                            with nc.allow_low_precision("fp32r recip"):
                                nc.vector.reciprocal(recip[64:65, :],
                                                     at[64:65, :])
                            # broadcast recip row to 64 rows via K=1 matmul
                            dps = ps2.tile([128, 512], F32, tag="dps", bufs=1)
                            nc.tensor.matmul(dps[0:64, :], ones[64:65, 0:64],
                                             recip[64:65, :],
                                             start=True, stop=True)
                            div = p2.tile([128, 512], F32, tag="dv", bufs=2)
                            nc.vector.tensor_copy(div[0:64, :], dps[0:64, :])
                            tcols = slice(tw * W + half * 512,
                                          tw * W + (half + 1) * 512)
                            if h % 2 == 0:
                                nc.vector.tensor_mul(attnT_t[hc][0:64, tcols],
                                                     at[0:64, :], div[0:64, :])
                            else:
                                tmp = p2.tile([64, 512], F32R, tag="tm",
                                              bufs=2)
                                nc.vector.tensor_mul(tmp[:, :], at[0:64, :],
                                                     div[0:64, :])
                                nc.sync.dma_start(attnT_t[hc][64:128, tcols],
                                                  tmp[:, :])

            # ---------------- phase 3: output projection ----------------
            with tc.tile_pool(name="ph3", bufs=1) as p3, \
                 tc.tile_pool(name="ps3", bufs=4, space="PSUM") as ps3:
                wo_t = []
                for kc in range(4):
                    wo = p3.tile([128, HID], F32R, tag=f"wo{kc}")
                    nc.sync.dma_start(wo[:, :], woT_d[kc * 128:(kc + 1) * 128, :])
                    wo_t.append(wo)
                for sc in range(SC):
                    ot = p3.tile([128, HID], F32, tag="ou", bufs=3)
                    for n in range(2):
                        po = ps3.tile([128, 512], F32, tag="p3")
                        for kc in range(4):
                            nc.tensor.matmul(
                                po[:, :],
                                attnT_t[kc][:, sc * 128:(sc + 1) * 128],
                                wo_t[kc][:, n * 512:(n + 1) * 512],
                                start=(kc == 0), stop=(kc == 3))
                        nc.vector.tensor_copy(ot[:, n * 512:(n + 1) * 512],
                                              po[:, :])
                    nc.sync.dma_start(out_d[sc * 128:(sc + 1) * 128, :], ot[:, :])

    nc.compile()
    return nc


def kernel(hidden_states, causal_mask, padding_mask,
           q_w, q_b, k_w, k_b, v_w, v_b, o_w, o_b):
    global _compiled
    from concourse.bass_utils import run_bass_kernel_spmd

    hidden_states = np.asarray(hidden_states, dtype=np.float32)
    padding_mask = np.asarray(padding_mask)
    q_w = np.asarray(q_w, dtype=np.float32)
    k_w = np.asarray(k_w, dtype=np.float32)
    v_w = np.asarray(v_w, dtype=np.float32)
    o_w = np.asarray(o_w, dtype=np.float32)
    q_b = np.asarray(q_b, dtype=np.float32)
    k_b = np.asarray(k_b, dtype=np.float32)
    v_b = np.asarray(v_b, dtype=np.float32)
    o_b = np.asarray(o_b, dtype=np.float32)

    if _compiled is None:
        _compiled = _build()
    nc = _compiled

    in_maps = []
    for b in range(BS):
        xT = np.ascontiguousarray(hidden_states[b].T)
        kmask = np.where(padding_mask[b], np.float32(-30000.0),
                         np.float32(0.0)).astype(np.float32)
        kmask2 = np.ascontiguousarray(kmask.reshape(SC, 128).T)
        for g in range(HG):
            r = slice(g * OG, (g + 1) * OG)
            in_maps.append({
                "xT": xT,
                "wqT": np.ascontiguousarray(q_w[r].T),
                "wkT": np.ascontiguousarray(k_w[r].T),
                "wvT": np.ascontiguousarray(v_w[r].T),
                "woT": np.ascontiguousarray(o_w[:, r].T),
                "qb": np.ascontiguousarray(q_b[r].reshape(4, 128).T),
                "kb": np.ascontiguousarray(k_b[r].reshape(4, 128).T),
                "vb": np.ascontiguousarray(v_b[r].reshape(1, OG)),
                "kmask": kmask2,
            })

    trace = os.environ.get("KERNEL_TRACE") == "1"
    res = run_bass_kernel_spmd(nc, in_maps, core_ids=list(range(NCORES)),
                               trace=trace)
    if trace and res.exec_time_ns is not None:
        print(f"HW exec time: {res.exec_time_ns} ns")
        if res.instructions_and_trace:
            print(f"trace: {res.instructions_and_trace[1]}")

    out = np.empty((BS, SEQ, HID), dtype=np.float32)
    for b in range(BS):
        out[b] = (res.results[2 * b]["out"] + res.results[2 * b + 1]["out"]
                  + o_b[None, :])
    return out



# revision 3
# speedup vs baseline: 1.4046x; 1.4046x over previous
"""Multi-head attention (bs=4, seq=2048, hidden=1024, 16 heads) on 8 trn2 cores.

Sharding: core = (batch b, head-group g) with 4 batches x 2 groups of 8 heads.
Each core computes QKV projections for its head slice, causal+padded softmax
attention, and a partial output projection; the host sums the two partial
outputs per batch and adds o_b.

v2 layout notes:
  - bf16 weights/activations in SBUF (fp32 accumulate in PSUM); host converts.
  - phase-1 projections run in four 512-seq quarters, emitted interleaved
    with the four 512-query attention windows so the Tile scheduler can keep
    the PE busy with projection matmuls while ScalarE runs softmax exps.
  - score matmuls for a head pair issue to disjoint 64-row PE groups
    (base partitions 0 and 64) so they execute concurrently.
  - exp runs once per (chunk, head-pair) as a single wide ACT op over a
    [128, 2, w] access pattern; padding mask rides as a per-partition bias.
  - softmax division: denominator rows leave PSUM via ScalarE ln, the
    reciprocal is exp(-ln) (same ACT table set), GpSimd broadcasts it
    across partitions, DVE does the final multiply.
"""
import os
import sys

for _p in ("/opt/trn_rl_repo",):
    if _p not in sys.path:
        sys.path.insert(0, _p)

import numpy as np

HID = 1024
HEADS = 16
D = 64
BS = 4
SEQ = 2048
NCORES = 8
HG = 2             # head groups (tensor-parallel axis)
HPG = HEADS // HG  # 8 heads per core
NPAIR = HPG // 2   # 4 head pairs per core
OG = HPG * D       # 512 projection dims per core
KC = HID // 128    # 8 hidden chunks
W = 512            # query window
NW = SEQ // W      # 4 windows (== phase-1 quarters)
SC = SEQ // 128    # 16 key chunks
SCALE = 1.0 / np.sqrt(D)

_compiled = None


def _build():
    import concourse.tile as tile
    from concourse import bacc, mybir

    F32 = mybir.dt.float32
    BF16 = mybir.dt.bfloat16
    AF = mybir.ActivationFunctionType
    Alu = mybir.AluOpType

    nc = bacc.Bacc("TRN2", target_bir_lowering=False, debug=False,
                   num_devices=NCORES)

    xT_d = nc.dram_tensor("xT", [HID, SEQ], BF16, kind="ExternalInput").ap()
    wqT_d = nc.dram_tensor("wqT", [HID, OG], BF16, kind="ExternalInput").ap()
    wkT_d = nc.dram_tensor("wkT", [HID, OG], BF16, kind="ExternalInput").ap()
    wvT_d = nc.dram_tensor("wvT", [HID, OG], BF16, kind="ExternalInput").ap()
    woT_d = nc.dram_tensor("woT", [OG, HID], BF16, kind="ExternalInput").ap()
    qb_d = nc.dram_tensor("qb", [128, 4], F32, kind="ExternalInput").ap()
    kb_d = nc.dram_tensor("kb", [128, 4], F32, kind="ExternalInput").ap()
    vb_d = nc.dram_tensor("vb", [1, OG], BF16, kind="ExternalInput").ap()
    kmask_d = nc.dram_tensor("kmask", [128, SC], F32, kind="ExternalInput").ap()
    out_d = nc.dram_tensor("out", [SEQ, HID], F32, kind="ExternalOutput").ap()

    with tile.TileContext(nc) as tc:
        with tc.tile_pool(name="const", bufs=1) as cp, \
             tc.tile_pool(name="wq", bufs=1) as wqp, \
             tc.tile_pool(name="wk", bufs=1) as wkp, \
             tc.tile_pool(name="wv", bufs=1) as wvp, \
             tc.tile_pool(name="wo", bufs=1) as wop, \
             tc.tile_pool(name="qT", bufs=1) as qTp, \
             tc.tile_pool(name="kT", bufs=1) as kTp, \
             tc.tile_pool(name="v", bufs=1) as vp, \
             tc.tile_pool(name="attnT", bufs=1) as aTp, \
             tc.tile_pool(name="x", bufs=2) as xp, \
             tc.tile_pool(name="ph2", bufs=1) as p2, \
             tc.tile_pool(name="ph3", bufs=1) as p3, \
             tc.tile_pool(name="psA", bufs=2, space="PSUM") as psA, \
             tc.tile_pool(name="psB", bufs=4, space="PSUM") as psB:

            # ---------------- constants ----------------
            ones_f = cp.tile([128, 128], F32, tag="ones_f")
            nc.gpsimd.memset(ones_f[:, :], 1.0)
            onesb = cp.tile([128, 128], BF16, tag="onesb")
            nc.scalar.copy(onesb[:, :], ones_f[:, :])
            # tri01[p, j] = 1 if j >= p else 0 (keep keys <= query), two
            # adjacent copies so one 3D-AP multiply masks both heads.
            tri01_f = cp.tile([128, 128], F32, tag="tri01_f")
            nc.gpsimd.affine_select(tri01_f[:, :], ones_f[:, :],
                                    pattern=[[1, 128]],
                                    compare_op=Alu.is_ge, fill=0.0,
                                    base=0, channel_multiplier=-1)
            tri2 = cp.tile([128, 256], BF16, tag="tri2")
            nc.scalar.copy(tri2[:, 0:128], tri01_f[:, :])
            nc.scalar.copy(tri2[:, 128:256], tri01_f[:, :])
            qb_s = cp.tile([128, 4], F32, tag="qb")
            nc.sync.dma_start(qb_s[:, :], qb_d[:, :])
            kb_s = cp.tile([128, 4], F32, tag="kb")
            nc.sync.dma_start(kb_s[:, :], kb_d[:, :])
            vb_s = cp.tile([1, OG], BF16, tag="vb")
            nc.sync.dma_start(vb_s[:, :], vb_d[:, :])
            kmask_s = cp.tile([128, SC], F32, tag="km")
            nc.sync.dma_start(kmask_s[:, :], kmask_d[:, :])

            # ---------------- weights (loaded once) ----------------
            wq_t, wk_t, wv_t = [], [], []
            for kc in range(KC):
                for lst, pool, src in ((wq_t, wqp, wqT_d), (wk_t, wkp, wkT_d),
                                       (wv_t, wvp, wvT_d)):
                    wt = pool.tile([128, OG], BF16, tag=f"w{kc}",
                                   name=f"w{len(lst)}_{kc}")
                    nc.sync.dma_start(wt[:, :],
                                      src[kc * 128:(kc + 1) * 128, :])
                    lst.append(wt)
            wo_t = []
            for kc in range(4):
                wo = wop.tile([128, HID], BF16, tag=f"wo{kc}")
                nc.sync.dma_start(wo[:, :], woT_d[kc * 128:(kc + 1) * 128, :])
                wo_t.append(wo)

            # ---------------- persistent activation tiles ----------------
            qT_t = [qTp.tile([128, SEQ], BF16, tag=f"qT{i}", name=f"qT{i}")
                    for i in range(NPAIR)]
            kT_t = [kTp.tile([128, SEQ], BF16, tag=f"kT{i}", name=f"kT{i}")
                    for i in range(NPAIR)]
            v_t = [vp.tile([128, HPG * 65], BF16, tag=f"v{i}", name=f"v{i}")
                   for i in range(SC)]
            for i in range(SC):
                vv = v_t[i].rearrange("p (h c) -> p h c", c=65)
                nc.gpsimd.memset(vv[:, :, 64:65], 1.0)
            attnT_t = [aTp.tile([128, SEQ], BF16, tag=f"aT{i}", name=f"aT{i}")
                       for i in range(NPAIR)]

            def phase1_quarter(q):
                qs = q * W
                xT_t = []
                for kc in range(KC):
                    xt = xp.tile([128, W], BF16, tag=f"xT{kc}",
                                 name=f"xT{q}_{kc}")
                    nc.sync.dma_start(
                        xt[:, :], xT_d[kc * 128:(kc + 1) * 128, qs:qs + W])
                    xT_t.append(xt)
                # Q/K projections: out partitions = proj dims, cols = seq
                for w_t, o_t, bias in ((wq_t, qT_t, qb_s), (wk_t, kT_t, kb_s)):
                    for oc in range(4):
                        pqk = psB.tile([128, W], F32, tag="b512", name="pqk")
                        for kc in range(KC):
                            nc.tensor.matmul(
                                pqk[:, :],
                                w_t[kc][:, oc * 128:(oc + 1) * 128],
                                xT_t[kc][:, :],
                                start=(kc == 0), stop=(kc == KC - 1))
                        nc.scalar.activation(
                            o_t[oc][:, qs:qs + W], pqk[:, :], AF.Identity,
                            bias=bias[:, oc:oc + 1], scale=1.0)
                # V projection: out partitions = seq chunk, cols = proj dims
                for sc in range(4):
                    scg = 4 * q + sc
                    pv = psB.tile([128, OG], F32, tag="b512", name="pv")
                    for kc in range(KC):
                        nc.tensor.matmul(
                            pv[:, :],
                            xT_t[kc][:, sc * 128:(sc + 1) * 128],
                            wv_t[kc][:, :],
                            start=(kc == 0), stop=False)
                    nc.tensor.matmul(pv[:, :], onesb[0:1, :], vb_s[0:1, :],
                                     start=False, stop=True)
                    src = pv.rearrange("p (h c) -> p h c", c=64)
                    dst = v_t[scg].rearrange("p (h c) -> p h c", c=65)
                    nc.scalar.activation(dst[:, :, 0:64], src[:, :, :],
                                         AF.Copy)

            def phase2_window(w):
                ws = w * W
                chunks = [(c, 0) for c in range(4 * w)]
                chunks += [(4 * w + i, 128 * i) for i in range(4)]
                last = len(chunks) - 1
                for pr in range(NPAIR):
                    he = 2 * pr       # even head (rows 0:64)
                    at_e = psB.tile([128, W], F32, tag="b512", name="at_e")
                    at_o = psB.tile([128, W], F32, tag="b512", name="at_o")
                    for idx, (c, off) in enumerate(chunks):
                        n = W - off
                        sp = psA.tile([128, 2 * W], F32, tag="sp", name="sp")
                        sp3 = sp.rearrange("p (g c) -> p g c", g=2)
                        nc.tensor.matmul(
                            sp[:, off:W],
                            kT_t[pr][0:64, c * 128:(c + 1) * 128],
                            qT_t[pr][0:64, ws + off:ws + W],
                            start=True, stop=True)
                        nc.tensor.matmul(
                            sp[:, W + off:2 * W],
                            kT_t[pr][64:128, c * 128:(c + 1) * 128],
                            qT_t[pr][64:128, ws + off:ws + W],
                            start=True, stop=True)
                        et = p2.tile([128, 2 * W], BF16, tag="E", bufs=4)
                        et3 = et.rearrange("p (g c) -> p g c", g=2)
                        nc.scalar.activation(et3[:, :, off:W],
                                             sp3[:, :, off:W], AF.Exp,
                                             bias=kmask_s[:, c:c + 1],
                                             scale=SCALE)
                        if off or c == 4 * w:  # diagonal chunk
                            nc.vector.tensor_mul(
                                et3[:, :, off:off + 128],
                                et3[:, :, off:off + 128],
                                tri2.rearrange("p (g c) -> p g c", g=2))
                        nc.tensor.matmul(
                            at_e[0:65, off:W],
                            v_t[c][:, he * 65:(he + 1) * 65],
                            et[:, off:W],
                            start=(idx == 0), stop=(idx == last))
                        nc.tensor.matmul(
                            at_o[0:65, off:W],
                            v_t[c][:, (he + 1) * 65:(he + 2) * 65],
                            et[:, W + off:2 * W],
                            start=(idx == 0), stop=(idx == last))
                    # softmax division, off the PE critical path:
                    # attnU = unnormalized att (bf16), denom -> 1/x via
                    # ln + exp(-x) on ScalarE, broadcast on GpSimd.
                    for h, at in ((he, at_e), (he + 1, at_o)):
                        attnU = p2.tile([64, W], BF16, tag="aU", bufs=3)
                        nc.vector.tensor_copy(attnU[:, :], at[0:64, :])
                        lnr = p2.tile([128, W], F32, tag="lnr", bufs=2)
                        nc.scalar.activation(lnr[64:65, :], at[64:65, :],
                                             AF.Ln)
                        rcp = p2.tile([128, W], BF16, tag="rcp", bufs=2)
                        nc.scalar.activation(rcp[64:65, :], lnr[64:65, :],
                                             AF.Exp, scale=-1.0)
                        # broadcast the reciprocal row to 64 partitions via
                        # a K=1 matmul (partition_broadcast ucode ignores a
                        # non-zero source base partition on HW)
                        dps = psB.tile([128, W], F32, tag="b512", name="dps")
                        nc.tensor.matmul(dps[0:64, :], onesb[64:65, 0:64],
                                         rcp[64:65, :], start=True, stop=True)
                        div = p2.tile([64, W], BF16, tag="dv", bufs=2)
                        nc.vector.tensor_copy(div[:, :], dps[0:64, :])
                        if h % 2 == 0:
                            nc.vector.tensor_mul(
                                attnT_t[pr][0:64, ws:ws + W],
                                attnU[:, :], div[:, :])
                        else:
                            tmp = p2.tile([64, W], BF16, tag="tm", bufs=2)
                            nc.vector.tensor_mul(tmp[:, :], attnU[:, :],
                                                 div[:, :])
                            nc.sync.dma_start(attnT_t[pr][64:128, ws:ws + W],
                                              tmp[:, :])

            # interleave projections quarters and attention windows so the
            # scheduler can overlap PE-bound and ScalarE-bound stretches
            for q in range(NW):
                phase1_quarter(q)
                phase2_window(q)

            # ---------------- phase 3: output projection ----------------
            for sc in range(SC):
                ot = p3.tile([128, HID], F32, tag="ou", bufs=3)
                for n in range(2):
                    po = psB.tile([128, W], F32, tag="b512", name="po")
                    for kc in range(4):
                        nc.tensor.matmul(
                            po[:, :],
                            attnT_t[kc][:, sc * 128:(sc + 1) * 128],
                            wo_t[kc][:, n * W:(n + 1) * W],
                            start=(kc == 0), stop=(kc == 3))
                    nc.vector.tensor_copy(ot[:, n * W:(n + 1) * W], po[:, :])
                nc.sync.dma_start(out_d[sc * 128:(sc + 1) * 128, :], ot[:, :])

    nc.compile()
    return nc


def kernel(hidden_states, causal_mask, padding_mask,
           q_w, q_b, k_w, k_b, v_w, v_b, o_w, o_b):
    global _compiled
    from concourse.bass_utils import run_bass_kernel_spmd
    import ml_dtypes

    BF = ml_dtypes.bfloat16

    hidden_states = np.asarray(hidden_states, dtype=np.float32)
    padding_mask = np.asarray(padding_mask)
    q_w = np.asarray(q_w, dtype=np.float32)
    k_w = np.asarray(k_w, dtype=np.float32)
    v_w = np.asarray(v_w, dtype=np.float32)
    o_w = np.asarray(o_w, dtype=np.float32)
    q_b = np.asarray(q_b, dtype=np.float32)
    k_b = np.asarray(k_b, dtype=np.float32)
    v_b = np.asarray(v_b, dtype=np.float32)
    o_b = np.asarray(o_b, dtype=np.float32)

    if _compiled is None:
        _compiled = _build()
    nc = _compiled

    in_maps = []
    for b in range(BS):
        xT = np.ascontiguousarray(hidden_states[b].T).astype(BF)
        kmask = np.where(padding_mask[b], np.float32(-30000.0),
                         np.float32(0.0)).astype(np.float32)
        kmask2 = np.ascontiguousarray(kmask.reshape(SC, 128).T)
        for g in range(HG):
            r = slice(g * OG, (g + 1) * OG)
            in_maps.append({
                "xT": xT,
                "wqT": np.ascontiguousarray(q_w[r].T).astype(BF),
                "wkT": np.ascontiguousarray(k_w[r].T).astype(BF),
                "wvT": np.ascontiguousarray(v_w[r].T).astype(BF),
                "woT": np.ascontiguousarray(o_w[:, r].T).astype(BF),
                "qb": np.ascontiguousarray(q_b[r].reshape(4, 128).T),
                "kb": np.ascontiguousarray(k_b[r].reshape(4, 128).T),
                "vb": np.ascontiguousarray(v_b[r].reshape(1, OG)).astype(BF),
                "kmask": kmask2,
            })

    trace = os.environ.get("KERNEL_TRACE") == "1"
    res = run_bass_kernel_spmd(nc, in_maps, core_ids=list(range(NCORES)),
                               trace=trace)
    if trace and res.exec_time_ns is not None:
        print(f"HW exec time: {res.exec_time_ns} ns")
        if res.instructions_and_trace:
            print(f"trace: {res.instructions_and_trace[1]}")

    out = np.empty((BS, SEQ, HID), dtype=np.float32)
    for b in range(BS):
        out[b] = (res.results[2 * b]["out"] + res.results[2 * b + 1]["out"]
                  + o_b[None, :])
    return out


# revision 8
# speedup vs baseline: 1.4200x; 1.0110x over previous
"""Multi-head attention (bs=4, seq=2048, hidden=1024, 16 heads) on 8 trn2 cores.

Sharding: core = (batch b, head-group g) with 4 batches x 2 groups of 8 heads.
Each core computes QKV projections for its head slice, causal+padded softmax
attention, and a partial output projection; the host sums the two partial
outputs per batch and adds o_b.

v2 layout notes:
  - bf16 weights/activations in SBUF (fp32 accumulate in PSUM); host converts.
  - phase-1 projections run in four 512-seq quarters, emitted interleaved
    with the four 512-query attention windows so the Tile scheduler can keep
    the PE busy with projection matmuls while ScalarE runs softmax exps.
  - score matmuls for a head pair issue to disjoint 64-row PE groups
    (base partitions 0 and 64) so they execute concurrently.
  - exp runs once per (chunk, head-pair) as a single wide ACT op over a
    [128, 2, w] access pattern; padding mask rides as a per-partition bias.
  - softmax division: denominator rows leave PSUM via ScalarE ln, the
    reciprocal is exp(-ln) (same ACT table set), GpSimd broadcasts it
    across partitions, DVE does the final multiply.
"""
import os
import sys

for _p in ("/opt/trn_rl_repo",):
    if _p not in sys.path:
        sys.path.insert(0, _p)

import numpy as np

HID = 1024
HEADS = 16
D = 64
BS = 4
SEQ = 2048
NCORES = 8
HG = 2             # head groups (tensor-parallel axis)
HPG = HEADS // HG  # 8 heads per core
NPAIR = HPG // 2   # 4 head pairs per core
OG = HPG * D       # 512 projection dims per core
KC = HID // 128    # 8 hidden chunks
W = 512            # query window
NW = SEQ // W      # 4 windows (== phase-1 quarters)
SC = SEQ // 128    # 16 key chunks
SCALE = 1.0 / np.sqrt(D)

_compiled = None


def _build():
    import concourse.tile as tile
    from concourse import bacc, mybir

    F32 = mybir.dt.float32
    BF16 = mybir.dt.bfloat16
    AF = mybir.ActivationFunctionType
    Alu = mybir.AluOpType

    nc = bacc.Bacc("TRN2", target_bir_lowering=False, debug=False,
                   num_devices=NCORES)

    xT_d = nc.dram_tensor("xT", [HID, SEQ], BF16, kind="ExternalInput").ap()
    wqT_d = nc.dram_tensor("wqT", [HID, OG], BF16, kind="ExternalInput").ap()
    wkT_d = nc.dram_tensor("wkT", [HID, OG], BF16, kind="ExternalInput").ap()
    wvT_d = nc.dram_tensor("wvT", [HID, OG], BF16, kind="ExternalInput").ap()
    woT_d = nc.dram_tensor("woT", [OG, HID], BF16, kind="ExternalInput").ap()
    qb_d = nc.dram_tensor("qb", [128, 4], F32, kind="ExternalInput").ap()
    kb_d = nc.dram_tensor("kb", [128, 4], F32, kind="ExternalInput").ap()
    vb_d = nc.dram_tensor("vb", [1, OG], BF16, kind="ExternalInput").ap()
    kmask_d = nc.dram_tensor("kmask", [128, SC], F32, kind="ExternalInput").ap()
    out_d = nc.dram_tensor("out", [SEQ, HID], F32, kind="ExternalOutput").ap()

    with tile.TileContext(nc) as tc:
        with tc.tile_pool(name="const", bufs=1) as cp, \
             tc.tile_pool(name="wq", bufs=1) as wqp, \
             tc.tile_pool(name="wk", bufs=1) as wkp, \
             tc.tile_pool(name="wv", bufs=1) as wvp, \
             tc.tile_pool(name="wo", bufs=1) as wop, \
             tc.tile_pool(name="qT", bufs=1) as qTp, \
             tc.tile_pool(name="kT", bufs=1) as kTp, \
             tc.tile_pool(name="v", bufs=1) as vp, \
             tc.tile_pool(name="attnT", bufs=1) as aTp, \
             tc.tile_pool(name="x", bufs=2) as xp, \
             tc.tile_pool(name="ph2", bufs=1) as p2, \
             tc.tile_pool(name="ph3", bufs=1) as p3, \
             tc.tile_pool(name="psA", bufs=2, space="PSUM") as psA, \
             tc.tile_pool(name="psB", bufs=4, space="PSUM") as psB:

            # ---------------- constants ----------------
            ones_f = cp.tile([128, 128], F32, tag="ones_f")
            nc.gpsimd.memset(ones_f[:, :], 1.0)
            onesb = cp.tile([128, 128], BF16, tag="onesb")
            nc.scalar.copy(onesb[:, :], ones_f[:, :])
            # tri01[p, j] = 1 if j >= p else 0 (keep keys <= query), two
            # adjacent copies so one 3D-AP multiply masks both heads.
            tri01_f = cp.tile([128, 128], F32, tag="tri01_f")
            nc.gpsimd.affine_select(tri01_f[:, :], ones_f[:, :],
                                    pattern=[[1, 128]],
                                    compare_op=Alu.is_ge, fill=0.0,
                                    base=0, channel_multiplier=-1)
            tri2 = cp.tile([128, 256], BF16, tag="tri2")
            nc.scalar.copy(tri2[:, 0:128], tri01_f[:, :])
            nc.scalar.copy(tri2[:, 128:256], tri01_f[:, :])
            qb_s = cp.tile([128, 4], F32, tag="qb")
            nc.sync.dma_start(qb_s[:, :], qb_d[:, :])
            kb_s = cp.tile([128, 4], F32, tag="kb")
            nc.sync.dma_start(kb_s[:, :], kb_d[:, :])
            vb_s = cp.tile([1, OG], BF16, tag="vb")
            nc.sync.dma_start(vb_s[:, :], vb_d[:, :])
            kmask_s = cp.tile([128, SC], F32, tag="km")
            nc.sync.dma_start(kmask_s[:, :], kmask_d[:, :])

            # ---------------- weights (loaded once) ----------------
            wq_t, wk_t, wv_t = [], [], []
            for kc in range(KC):
                for lst, pool, src in ((wq_t, wqp, wqT_d), (wk_t, wkp, wkT_d),
                                       (wv_t, wvp, wvT_d)):
                    wt = pool.tile([128, OG], BF16, tag=f"w{kc}",
                                   name=f"w{len(lst)}_{kc}")
                    nc.sync.dma_start(wt[:, :],
                                      src[kc * 128:(kc + 1) * 128, :])
                    lst.append(wt)
            wo_t = []
            for kc in range(4):
                wo = wop.tile([128, HID], BF16, tag=f"wo{kc}")
                nc.sync.dma_start(wo[:, :], woT_d[kc * 128:(kc + 1) * 128, :])
                wo_t.append(wo)

            # ---------------- persistent activation tiles ----------------
            qT_t = [qTp.tile([128, SEQ], BF16, tag=f"qT{i}", name=f"qT{i}")
                    for i in range(NPAIR)]
            kT_t = [kTp.tile([128, SEQ], BF16, tag=f"kT{i}", name=f"kT{i}")
                    for i in range(NPAIR)]
            v_t = [vp.tile([128, HPG * 65], BF16, tag=f"v{i}", name=f"v{i}")
                   for i in range(SC)]
            for i in range(SC):
                vv = v_t[i].rearrange("p (h c) -> p h c", c=65)
                nc.gpsimd.memset(vv[:, :, 64:65], 1.0)
            attnT_t = [aTp.tile([128, SEQ], BF16, tag=f"aT{i}", name=f"aT{i}")
                       for i in range(NPAIR)]

            def phase1_quarter(q):
                qs = q * W
                xT_t = []
                for kc in range(KC):
                    xt = xp.tile([128, W], BF16, tag=f"xT{kc}",
                                 name=f"xT{q}_{kc}")
                    nc.sync.dma_start(
                        xt[:, :], xT_d[kc * 128:(kc + 1) * 128, qs:qs + W])
                    xT_t.append(xt)
                # Q/K projections: out partitions = proj dims, cols = seq
                for w_t, o_t, bias in ((wq_t, qT_t, qb_s), (wk_t, kT_t, kb_s)):
                    for oc in range(4):
                        pqk = psB.tile([128, W], F32, tag="b512", name="pqk")
                        for kc in range(KC):
                            nc.tensor.matmul(
                                pqk[:, :],
                                w_t[kc][:, oc * 128:(oc + 1) * 128],
                                xT_t[kc][:, :],
                                start=(kc == 0), stop=(kc == KC - 1))
                        nc.vector.tensor_scalar_add(
                            o_t[oc][:, qs:qs + W], pqk[:, :],
                            bias[:, oc:oc + 1])
                # V projection: out partitions = seq chunk, cols = proj dims
                for sc in range(4):
                    scg = 4 * q + sc
                    pv = psB.tile([128, OG], F32, tag="b512", name="pv")
                    for kc in range(KC):
                        nc.tensor.matmul(
                            pv[:, :],
                            xT_t[kc][:, sc * 128:(sc + 1) * 128],
                            wv_t[kc][:, :],
                            start=(kc == 0), stop=False)
                    nc.tensor.matmul(pv[:, :], onesb[0:1, :], vb_s[0:1, :],
                                     start=False, stop=True)
                    src = pv.rearrange("p (h c) -> p h c", c=64)
                    dst = v_t[scg].rearrange("p (h c) -> p h c", c=65)
                    nc.vector.tensor_copy(dst[:, :, 0:64], src[:, :, :])

            def phase2_window(w):
                ws = w * W
                chunks = [(c, 0) for c in range(4 * w)]
                chunks += [(4 * w + i, 128 * i) for i in range(4)]
                last = len(chunks) - 1
                for pr in range(NPAIR):
                    he = 2 * pr       # even head (rows 0:64)
                    at_e = psB.tile([128, W], F32, tag="b512", name="at_e")
                    at_o = psB.tile([128, W], F32, tag="b512", name="at_o")
                    for idx, (c, off) in enumerate(chunks):
                        n = W - off
                        sp = psA.tile([128, 2 * W], F32, tag="sp", name="sp")
                        sp3 = sp.rearrange("p (g c) -> p g c", g=2)
                        nc.tensor.matmul(
                            sp[:, off:W],
                            kT_t[pr][0:64, c * 128:(c + 1) * 128],
                            qT_t[pr][0:64, ws + off:ws + W],
                            start=True, stop=True)
                        nc.tensor.matmul(
                            sp[:, W + off:2 * W],
                            kT_t[pr][64:128, c * 128:(c + 1) * 128],
                            qT_t[pr][64:128, ws + off:ws + W],
                            start=True, stop=True)
                        et = p2.tile([128, 2 * W], BF16, tag="E", bufs=4)
                        et3 = et.rearrange("p (g c) -> p g c", g=2)
                        nc.scalar.activation(et3[:, :, off:W],
                                             sp3[:, :, off:W], AF.Exp,
                                             bias=kmask_s[:, c:c + 1],
                                             scale=SCALE)
                        if off or c == 4 * w:  # diagonal chunk
                            nc.vector.tensor_mul(
                                et3[:, :, off:off + 128],
                                et3[:, :, off:off + 128],
                                tri2.rearrange("p (g c) -> p g c", g=2))
                        nc.tensor.matmul(
                            at_e[0:65, off:W],
                            v_t[c][:, he * 65:(he + 1) * 65],
                            et[:, off:W],
                            start=(idx == 0), stop=(idx == last))
                        nc.tensor.matmul(
                            at_o[0:65, off:W],
                            v_t[c][:, (he + 1) * 65:(he + 2) * 65],
                            et[:, W + off:2 * W],
                            start=(idx == 0), stop=(idx == last))
                    # softmax division, off the PE critical path:
                    # evict unnormalized att + denominator row quickly
                    # (frees the PSUM slot), reciprocal on idle GpSimd,
                    # broadcast across partitions via a K=1 matmul.
                    for h, at in ((he, at_e), (he + 1, at_o)):
                        attnU = p2.tile([64, W], BF16, tag="aU", bufs=3)
                        nc.vector.tensor_copy(attnU[:, :], at[0:64, :])
                        dnr = p2.tile([128, W], F32, tag="dnr", bufs=2)
                        nc.scalar.copy(dnr[64:65, :], at[64:65, :])
                        # reciprocal: DVE divide costs 8 cyc per FREE elem,
                        # so reshape the row to [128, 4] via DMA first
                        dnT = p2.tile([128, 4], F32, tag="dnT", bufs=2)
                        nc.sync.dma_start(dnT[:, :], dnr[64:65, :])
                        dnTr = p2.tile([128, 4], BF16, tag="dnTr", bufs=2)
                        with nc.allow_low_precision("recip"):
                            nc.vector.reciprocal(dnTr[:, :], dnT[:, :])
                        rcp = p2.tile([128, W], BF16, tag="rcp", bufs=2)
                        nc.sync.dma_start(rcp[64:65, :], dnTr[:, :])
                        dps = psB.tile([128, W], F32, tag="b512", name="dps")
                        nc.tensor.matmul(dps[0:64, :], onesb[64:65, 0:64],
                                         rcp[64:65, :], start=True, stop=True)
                        if h % 2 == 0:
                            nc.vector.tensor_mul(
                                attnT_t[pr][0:64, ws:ws + W],
                                attnU[:, :], dps[0:64, :])
                        else:
                            tmp = p2.tile([64, W], BF16, tag="tm", bufs=2)
                            nc.vector.tensor_mul(tmp[:, :], attnU[:, :],
                                                 dps[0:64, :])
                            nc.sync.dma_start(attnT_t[pr][64:128, ws:ws + W],
                                              tmp[:, :])

            # interleave projections quarters and attention windows so the
            # scheduler can overlap PE-bound and ScalarE-bound stretches
            for q in range(NW):
                phase1_quarter(q)
                phase2_window(q)

            # ---------------- phase 3: output projection ----------------
            for sc in range(SC):
                ot = p3.tile([128, HID], F32, tag="ou", bufs=3)
                for n in range(2):
                    po = psB.tile([128, W], F32, tag="b512", name="po")
                    for kc in range(4):
                        nc.tensor.matmul(
                            po[:, :],
                            attnT_t[kc][:, sc * 128:(sc + 1) * 128],
                            wo_t[kc][:, n * W:(n + 1) * W],
                            start=(kc == 0), stop=(kc == 3))
                    nc.vector.tensor_copy(ot[:, n * W:(n + 1) * W], po[:, :])
                nc.sync.dma_start(out_d[sc * 128:(sc + 1) * 128, :], ot[:, :])

    nc.compile()
    return nc


def kernel(hidden_states, causal_mask, padding_mask,
           q_w, q_b, k_w, k_b, v_w, v_b, o_w, o_b):
    global _compiled
    from concourse.bass_utils import run_bass_kernel_spmd
    import ml_dtypes

    BF = ml_dtypes.bfloat16

    hidden_states = np.asarray(hidden_states, dtype=np.float32)
    padding_mask = np.asarray(padding_mask)
    q_w = np.asarray(q_w, dtype=np.float32)
    k_w = np.asarray(k_w, dtype=np.float32)
    v_w = np.asarray(v_w, dtype=np.float32)
    o_w = np.asarray(o_w, dtype=np.float32)
    q_b = np.asarray(q_b, dtype=np.float32)
    k_b = np.asarray(k_b, dtype=np.float32)
    v_b = np.asarray(v_b, dtype=np.float32)
    o_b = np.asarray(o_b, dtype=np.float32)

    if _compiled is None:
        _compiled = _build()
    nc = _compiled

    in_maps = []
    for b in range(BS):
        xT = np.ascontiguousarray(hidden_states[b].T).astype(BF)
        kmask = np.where(padding_mask[b], np.float32(-30000.0),
                         np.float32(0.0)).astype(np.float32)
        kmask2 = np.ascontiguousarray(kmask.reshape(SC, 128).T)
        for g in range(HG):
            r = slice(g * OG, (g + 1) * OG)
            in_maps.append({
                "xT": xT,
                "wqT": np.ascontiguousarray(q_w[r].T).astype(BF),
                "wkT": np.ascontiguousarray(k_w[r].T).astype(BF),
                "wvT": np.ascontiguousarray(v_w[r].T).astype(BF),
                "woT": np.ascontiguousarray(o_w[:, r].T).astype(BF),
                "qb": np.ascontiguousarray(q_b[r].reshape(4, 128).T),
                "kb": np.ascontiguousarray(k_b[r].reshape(4, 128).T),
                "vb": np.ascontiguousarray(v_b[r].reshape(1, OG)).astype(BF),
                "kmask": kmask2,
            })

    trace = os.environ.get("KERNEL_TRACE") == "1"
    res = run_bass_kernel_spmd(nc, in_maps, core_ids=list(range(NCORES)),
                               trace=trace)
    if trace and res.exec_time_ns is not None:
        print(f"HW exec time: {res.exec_time_ns} ns")
        if res.instructions_and_trace:
            print(f"trace: {res.instructions_and_trace[1]}")

    out = np.empty((BS, SEQ, HID), dtype=np.float32)
    for b in range(BS):
        out[b] = (res.results[2 * b]["out"] + res.results[2 * b + 1]["out"]
                  + o_b[None, :])
    return out


# revision 15
# speedup vs baseline: 1.7101x; 1.2043x over previous
"""Multi-head attention (bs=4, seq=2048, hidden=1024, 16 heads) on 8 trn2 cores.

Sharding: core = (batch b, head-group g) with 4 batches x 2 groups of 8 heads.
Each core computes QKV projections for its head slice, causal+padded softmax
attention, and a partial output projection; the host sums the two partial
outputs per batch and adds o_b.

v2 layout notes:
  - bf16 weights/activations in SBUF (fp32 accumulate in PSUM); host converts.
  - phase-1 projections run in four 512-seq quarters, emitted interleaved
    with the four 512-query attention windows so the Tile scheduler can keep
    the PE busy with projection matmuls while ScalarE runs softmax exps.
  - score matmuls for a head pair issue to disjoint 64-row PE groups
    (base partitions 0 and 64) so they execute concurrently.
  - exp runs once per (chunk, head-pair) as a single wide ACT op over a
    [128, 2, w] access pattern; padding mask rides as a per-partition bias.
  - softmax division: denominator rows leave PSUM via ScalarE ln, the
    reciprocal is exp(-ln) (same ACT table set), GpSimd broadcasts it
    across partitions, DVE does the final multiply.
"""
import os
import sys

for _p in ("/opt/trn_rl_repo",):
    if _p not in sys.path:
        sys.path.insert(0, _p)

import numpy as np

HID = 1024
HEADS = 16
D = 64
BS = 4
SEQ = 2048
NCORES = 8
HG = 2             # head groups (tensor-parallel axis)
HPG = HEADS // HG  # 8 heads per core
NPAIR = HPG // 2   # 4 head pairs per core
OG = HPG * D       # 512 projection dims per core
KC = HID // 128    # 8 hidden chunks
W = 512            # query window
NW = SEQ // W      # 4 windows (== phase-1 quarters)
SC = SEQ // 128    # 16 key chunks
SCALE = 1.0 / np.sqrt(D)

_compiled = None


def _build():
    import concourse.tile as tile
    from concourse import bacc, mybir

    F32 = mybir.dt.float32
    BF16 = mybir.dt.bfloat16
    AF = mybir.ActivationFunctionType
    Alu = mybir.AluOpType

    nc = bacc.Bacc("TRN2", target_bir_lowering=False, debug=False,
                   num_devices=NCORES)

    xT_d = nc.dram_tensor("xT", [HID, SEQ], BF16, kind="ExternalInput").ap()
    wqT_d = nc.dram_tensor("wqT", [HID, OG], BF16, kind="ExternalInput").ap()
    wkT_d = nc.dram_tensor("wkT", [HID, OG], BF16, kind="ExternalInput").ap()
    wvT_d = nc.dram_tensor("wvT", [HID, OG], BF16, kind="ExternalInput").ap()
    woT_d = nc.dram_tensor("woT", [OG, HID], BF16, kind="ExternalInput").ap()
    qb_d = nc.dram_tensor("qb", [128, 4], F32, kind="ExternalInput").ap()
    kb_d = nc.dram_tensor("kb", [128, 4], F32, kind="ExternalInput").ap()
    vb_d = nc.dram_tensor("vb", [1, OG], BF16, kind="ExternalInput").ap()
    kmask_d = nc.dram_tensor("kmask", [128, SC], F32, kind="ExternalInput").ap()
    out_d = nc.dram_tensor("out", [SEQ, HID], F32, kind="ExternalOutput").ap()

    with tile.TileContext(nc) as tc:
        with tc.tile_pool(name="const", bufs=1) as cp, \
             tc.tile_pool(name="wq", bufs=1) as wqp, \
             tc.tile_pool(name="wk", bufs=1) as wkp, \
             tc.tile_pool(name="wv", bufs=1) as wvp, \
             tc.tile_pool(name="wo", bufs=1) as wop, \
             tc.tile_pool(name="qT", bufs=1) as qTp, \
             tc.tile_pool(name="kT", bufs=1) as kTp, \
             tc.tile_pool(name="v", bufs=1) as vp, \
             tc.tile_pool(name="attnT", bufs=1) as aTp, \
             tc.tile_pool(name="x", bufs=2) as xp, \
             tc.tile_pool(name="ph2", bufs=1) as p2, \
             tc.tile_pool(name="ph3", bufs=1) as p3, \
             tc.tile_pool(name="psA", bufs=2, space="PSUM") as psA, \
             tc.tile_pool(name="psB", bufs=3, space="PSUM") as psB, \
             tc.tile_pool(name="psC", bufs=1, space="PSUM") as psC:

            # ---------------- constants ----------------
            ones_f = cp.tile([128, 128], F32, tag="ones_f")
            nc.gpsimd.memset(ones_f[:, :], 1.0)
            onesb = cp.tile([128, 128], BF16, tag="onesb")
            nc.scalar.copy(onesb[:, :], ones_f[:, :])
            # tri01[p, j] = 1 if j >= p else 0 (keep keys <= query), two
            # adjacent copies so one 3D-AP multiply masks both heads.
            tri01_f = cp.tile([128, 128], F32, tag="tri01_f")
            nc.gpsimd.affine_select(tri01_f[:, :], ones_f[:, :],
                                    pattern=[[1, 128]],
                                    compare_op=Alu.is_ge, fill=0.0,
                                    base=0, channel_multiplier=-1)
            tri2 = cp.tile([128, 256], BF16, tag="tri2")
            nc.scalar.copy(tri2[:, 0:128], tri01_f[:, :])
            nc.scalar.copy(tri2[:, 128:256], tri01_f[:, :])
            qb_s = cp.tile([128, 4], F32, tag="qb")
            nc.sync.dma_start(qb_s[:, :], qb_d[:, :])
            kb_s = cp.tile([128, 4], F32, tag="kb")
            nc.sync.dma_start(kb_s[:, :], kb_d[:, :])
            vb_s = cp.tile([1, OG], BF16, tag="vb")
            nc.sync.dma_start(vb_s[:, :], vb_d[:, :])
            kmask_s = cp.tile([128, SC], F32, tag="km")
            nc.sync.dma_start(kmask_s[:, :], kmask_d[:, :])

            # ---------------- weights (loaded once) ----------------
            wq_t, wk_t, wv_t = [], [], []
            for kc in range(KC):
                for lst, pool, src in ((wq_t, wqp, wqT_d), (wk_t, wkp, wkT_d),
                                       (wv_t, wvp, wvT_d)):
                    wt = pool.tile([128, OG], BF16, tag=f"w{kc}",
                                   name=f"w{len(lst)}_{kc}")
                    nc.sync.dma_start(wt[:, :],
                                      src[kc * 128:(kc + 1) * 128, :])
                    lst.append(wt)
            wo_t = []
            for kc in range(4):
                wo = wop.tile([128, HID], BF16, tag=f"wo{kc}")
                nc.sync.dma_start(wo[:, :], woT_d[kc * 128:(kc + 1) * 128, :])
                wo_t.append(wo)

            # ---------------- persistent activation tiles ----------------
            qT_t = [qTp.tile([128, SEQ], BF16, tag=f"qT{i}", name=f"qT{i}")
                    for i in range(NPAIR)]
            kT_t = [kTp.tile([128, SEQ], BF16, tag=f"kT{i}", name=f"kT{i}")
                    for i in range(NPAIR)]
            v_t = [vp.tile([128, HPG * 65], BF16, tag=f"v{i}", name=f"v{i}")
                   for i in range(SC)]
            for i in range(SC):
                vv = v_t[i].rearrange("p (h c) -> p h c", c=65)
                nc.gpsimd.memset(vv[:, :, 64:65], 1.0)
            attnT_t = [aTp.tile([128, SEQ], BF16, tag=f"aT{i}", name=f"aT{i}")
                       for i in range(NPAIR)]

            def phase1_quarter(q):
                qs = q * W
                xT_t = []
                for kc in range(KC):
                    xt = xp.tile([128, W], BF16, tag=f"xT{kc}",
                                 name=f"xT{q}_{kc}")
                    nc.sync.dma_start(
                        xt[:, :], xT_d[kc * 128:(kc + 1) * 128, qs:qs + W])
                    xT_t.append(xt)
                # Q/K projections: out partitions = proj dims, cols = seq
                for w_t, o_t, bias in ((wq_t, qT_t, qb_s), (wk_t, kT_t, kb_s)):
                    for oc in range(4):
                        pqk = psC.tile([128, W], F32, tag="c512", name="pqk")
                        for kc in range(KC):
                            nc.tensor.matmul(
                                pqk[:, :],
                                w_t[kc][:, oc * 128:(oc + 1) * 128],
                                xT_t[kc][:, :],
                                start=(kc == 0), stop=(kc == KC - 1))
                        nc.vector.tensor_scalar_add(
                            o_t[oc][:, qs:qs + W], pqk[:, :],
                            bias[:, oc:oc + 1])
                # V projection: out partitions = seq chunk, cols = proj dims
                for sc in range(4):
                    scg = 4 * q + sc
                    pv = psC.tile([128, OG], F32, tag="c512", name="pv")
                    for kc in range(KC):
                        nc.tensor.matmul(
                            pv[:, :],
                            xT_t[kc][:, sc * 128:(sc + 1) * 128],
                            wv_t[kc][:, :],
                            start=(kc == 0), stop=False)
                    nc.tensor.matmul(pv[:, :], onesb[0:1, :], vb_s[0:1, :],
                                     start=False, stop=True)
                    src = pv.rearrange("p (h c) -> p h c", c=64)
                    dst = v_t[scg].rearrange("p (h c) -> p h c", c=65)
                    nc.vector.tensor_copy(dst[:, :, 0:64], src[:, :, :])

            def phase2_window(w):
                ws = w * W
                chunks = [(c, 0) for c in range(4 * w)]
                chunks += [(4 * w + i, 128 * i) for i in range(4)]
                last = len(chunks) - 1
                for pr in range(NPAIR):
                    he = 2 * pr       # even head (rows 0:64)
                    at_e = psB.tile([128, W], F32, tag="b512", name="at_e")
                    at_o = psB.tile([128, W], F32, tag="b512", name="at_o")
                    for idx, (c, off) in enumerate(chunks):
                        n = W - off
                        sp = psA.tile([128, 2 * W], F32, tag="sp", name="sp")
                        sp3 = sp.rearrange("p (g c) -> p g c", g=2)
                        nc.tensor.matmul(
                            sp[:, off:W],
                            kT_t[pr][0:64, c * 128:(c + 1) * 128],
                            qT_t[pr][0:64, ws + off:ws + W],
                            start=True, stop=True)
                        nc.tensor.matmul(
                            sp[:, W + off:2 * W],
                            kT_t[pr][64:128, c * 128:(c + 1) * 128],
                            qT_t[pr][64:128, ws + off:ws + W],
                            start=True, stop=True)
                        et = p2.tile([128, 2 * W], BF16, tag="E", bufs=6)
                        et3 = et.rearrange("p (g c) -> p g c", g=2)
                        nc.scalar.activation(et3[:, :, off:W],
                                             sp3[:, :, off:W], AF.Exp,
                                             bias=kmask_s[:, c:c + 1],
                                             scale=SCALE)
                        if off or c == 4 * w:  # diagonal chunk
                            nc.vector.tensor_mul(
                                et3[:, :, off:off + 128],
                                et3[:, :, off:off + 128],
                                tri2.rearrange("p (g c) -> p g c", g=2))
                        nc.tensor.matmul(
                            at_e[0:65, off:W],
                            v_t[c][:, he * 65:(he + 1) * 65],
                            et[:, off:W],
                            start=(idx == 0), stop=(idx == last))
                        nc.tensor.matmul(
                            at_o[0:65, off:W],
                            v_t[c][:, (he + 1) * 65:(he + 2) * 65],
                            et[:, W + off:2 * W],
                            start=(idx == 0), stop=(idx == last))
                    # softmax division, off the PE critical path:
                    # evict unnormalized att + denominator row quickly
                    # (frees the PSUM slot), reciprocal on idle GpSimd,
                    # broadcast across partitions via a K=1 matmul.
                    for h, at in ((he, at_e), (he + 1, at_o)):
                        attnU = p2.tile([64, W], BF16, tag="aU", bufs=3)
                        nc.vector.tensor_copy(attnU[:, :], at[0:64, :])
                        dnr = p2.tile([128, W], F32, tag="dnr", bufs=2)
                        nc.vector.tensor_copy(dnr[64:65, :], at[64:65, :])
                        # reciprocal: DVE divide costs 8 cyc per FREE elem,
                        # so reshape the row to [128, 4] via DMA first
                        dnT = p2.tile([128, 4], F32, tag="dnT", bufs=2)
                        nc.sync.dma_start(dnT[:, :], dnr[64:65, :])
                        dnTr = p2.tile([128, 4], BF16, tag="dnTr", bufs=2)
                        with nc.allow_low_precision("recip"):
                            nc.vector.reciprocal(dnTr[:, :], dnT[:, :])
                        rcp = p2.tile([128, W], BF16, tag="rcp", bufs=2)
                        nc.sync.dma_start(rcp[64:65, :], dnTr[:, :])
                        # broadcast the reciprocal row back into the (now
                        # dead) at tile -- reuses its PSUM bank, WAW-ordered
                        # behind the two evictions above
                        nc.tensor.matmul(at[0:64, :], onesb[64:65, 0:64],
                                         rcp[64:65, :], start=True, stop=True)
                        if h % 2 == 0:
                            nc.vector.tensor_mul(
                                attnT_t[pr][0:64, ws:ws + W],
                                attnU[:, :], at[0:64, :])
                        else:
                            tmp = p2.tile([64, W], BF16, tag="tm", bufs=2)
                            nc.vector.tensor_mul(tmp[:, :], attnU[:, :],
                                                 at[0:64, :])
                            nc.sync.dma_start(attnT_t[pr][64:128, ws:ws + W],
                                              tmp[:, :])

            def phase3_window(w):
                # output projection for the sq chunks of window w
                for sc in range(4 * w, 4 * w + 4):
                    ot = p3.tile([128, HID], F32, tag="ou", bufs=3)
                    for n in range(2):
                        po = psC.tile([128, W], F32, tag="c512", name="po")
                        for kc in range(4):
                            nc.tensor.matmul(
                                po[:, :],
                                attnT_t[kc][:, sc * 128:(sc + 1) * 128],
                                wo_t[kc][:, n * W:(n + 1) * W],
                                start=(kc == 0), stop=(kc == 3))
                        nc.vector.tensor_copy(ot[:, n * W:(n + 1) * W],
                                              po[:, :])
                    nc.sync.dma_start(out_d[sc * 128:(sc + 1) * 128, :],
                                      ot[:, :])

            # interleave projection quarters, attention windows, and output
            # projection so the scheduler can fill PE idle time during
            # ScalarE-bound (softmax) stretches
            phase1_quarter(0)
            phase2_window(0)
            for q in range(1, NW):
                phase1_quarter(q)
                phase2_window(q)
                phase3_window(q - 1)
            phase3_window(NW - 1)

    nc.compile()
    return nc


def kernel(hidden_states, causal_mask, padding_mask,
           q_w, q_b, k_w, k_b, v_w, v_b, o_w, o_b):
    global _compiled
    from concourse.bass_utils import run_bass_kernel_spmd
    import ml_dtypes

    BF = ml_dtypes.bfloat16

    hidden_states = np.asarray(hidden_states, dtype=np.float32)
    padding_mask = np.asarray(padding_mask)
    q_w = np.asarray(q_w, dtype=np.float32)
    k_w = np.asarray(k_w, dtype=np.float32)
    v_w = np.asarray(v_w, dtype=np.float32)
    o_w = np.asarray(o_w, dtype=np.float32)
    q_b = np.asarray(q_b, dtype=np.float32)
    k_b = np.asarray(k_b, dtype=np.float32)
    v_b = np.asarray(v_b, dtype=np.float32)
    o_b = np.asarray(o_b, dtype=np.float32)

    if _compiled is None:
        _compiled = _build()
    nc = _compiled

    in_maps = []
    for b in range(BS):
        xT = np.ascontiguousarray(hidden_states[b].T).astype(BF)
        kmask = np.where(padding_mask[b], np.float32(-30000.0),
                         np.float32(0.0)).astype(np.float32)
        kmask2 = np.ascontiguousarray(kmask.reshape(SC, 128).T)
        for g in range(HG):
            r = slice(g * OG, (g + 1) * OG)
            in_maps.append({
                "xT": xT,
                "wqT": np.ascontiguousarray(q_w[r].T).astype(BF),
                "wkT": np.ascontiguousarray(k_w[r].T).astype(BF),
                "wvT": np.ascontiguousarray(v_w[r].T).astype(BF),
                "woT": np.ascontiguousarray(o_w[:, r].T).astype(BF),
                "qb": np.ascontiguousarray(q_b[r].reshape(4, 128).T),
                "kb": np.ascontiguousarray(k_b[r].reshape(4, 128).T),
                "vb": np.ascontiguousarray(v_b[r].reshape(1, OG)).astype(BF),
                "kmask": kmask2,
            })

    trace = os.environ.get("KERNEL_TRACE") == "1"
    res = run_bass_kernel_spmd(nc, in_maps, core_ids=list(range(NCORES)),
                               trace=trace)
    if trace and res.exec_time_ns is not None:
        print(f"HW exec time: {res.exec_time_ns} ns")
        if res.instructions_and_trace:
            print(f"trace: {res.instructions_and_trace[1]}")

    out = np.empty((BS, SEQ, HID), dtype=np.float32)
    for b in range(BS):
        out[b] = (res.results[2 * b]["out"] + res.results[2 * b + 1]["out"]
                  + o_b[None, :])
    return out


# revision 21
# speedup vs baseline: 1.7899x; 1.0467x over previous
"""Multi-head attention (bs=4, seq=2048, hidden=1024, 16 heads) on 8 trn2 cores.

Sharding: core = (batch b, head-group g) with 4 batches x 2 groups of 8 heads.
Each core computes QKV projections for its head slice, causal+padded softmax
attention, and a partial output projection; the host sums the two partial
outputs per batch and adds o_b.

v2 layout notes:
  - bf16 weights/activations in SBUF (fp32 accumulate in PSUM); host converts.
  - phase-1 projections run in four 512-seq quarters, emitted interleaved
    with the four 512-query attention windows so the Tile scheduler can keep
    the PE busy with projection matmuls while ScalarE runs softmax exps.
  - score matmuls for a head pair issue to disjoint 64-row PE groups
    (base partitions 0 and 64) so they execute concurrently.
  - exp runs once per (chunk, head-pair) as a single wide ACT op over a
    [128, 2, w] access pattern; padding mask rides as a per-partition bias.
  - softmax division: denominator rows leave PSUM via ScalarE ln, the
    reciprocal is exp(-ln) (same ACT table set), GpSimd broadcasts it
    across partitions, DVE does the final multiply.
"""
import os
import sys

for _p in ("/opt/trn_rl_repo",):
    if _p not in sys.path:
        sys.path.insert(0, _p)

import numpy as np

HID = 1024
HEADS = 16
D = 64
BS = 4
SEQ = 2048
NCORES = 8
HG = 2             # head groups (tensor-parallel axis)
HPG = HEADS // HG  # 8 heads per core
NPAIR = HPG // 2   # 4 head pairs per core
OG = HPG * D       # 512 projection dims per core
KC = HID // 128    # 8 hidden chunks
W = 512            # query window
NW = SEQ // W      # 4 windows (== phase-1 quarters)
SC = SEQ // 128    # 16 key chunks
SCALE = 1.0 / np.sqrt(D)

_compiled = None


def _build():
    import concourse.tile as tile
    from concourse import bacc, mybir

    F32 = mybir.dt.float32
    BF16 = mybir.dt.bfloat16
    AF = mybir.ActivationFunctionType
    Alu = mybir.AluOpType

    nc = bacc.Bacc("TRN2", target_bir_lowering=False, debug=False,
                   num_devices=NCORES)

    xT_d = nc.dram_tensor("xT", [HID, SEQ], BF16, kind="ExternalInput").ap()
    wqT_d = nc.dram_tensor("wqT", [HID, OG], BF16, kind="ExternalInput").ap()
    wkT_d = nc.dram_tensor("wkT", [HID, OG], BF16, kind="ExternalInput").ap()
    wvT_d = nc.dram_tensor("wvT", [HID, OG], BF16, kind="ExternalInput").ap()
    woT_d = nc.dram_tensor("woT", [OG, HID], BF16, kind="ExternalInput").ap()
    qb_d = nc.dram_tensor("qb", [128, 4], F32, kind="ExternalInput").ap()
    kb_d = nc.dram_tensor("kb", [128, 4], F32, kind="ExternalInput").ap()
    vb_d = nc.dram_tensor("vb", [1, OG], BF16, kind="ExternalInput").ap()
    kmask_d = nc.dram_tensor("kmask", [128, SC], F32, kind="ExternalInput").ap()
    out_d = nc.dram_tensor("out", [SEQ, HID], F32, kind="ExternalOutput").ap()

    with tile.TileContext(nc) as tc:
        with tc.tile_pool(name="const", bufs=1) as cp, \
             tc.tile_pool(name="wq", bufs=1) as wqp, \
             tc.tile_pool(name="wk", bufs=1) as wkp, \
             tc.tile_pool(name="wv", bufs=1) as wvp, \
             tc.tile_pool(name="wo", bufs=1) as wop, \
             tc.tile_pool(name="qT", bufs=1) as qTp, \
             tc.tile_pool(name="kT", bufs=1) as kTp, \
             tc.tile_pool(name="v", bufs=1) as vp, \
             tc.tile_pool(name="attnT", bufs=1) as aTp, \
             tc.tile_pool(name="x", bufs=2) as xp, \
             tc.tile_pool(name="ph2", bufs=1) as p2, \
             tc.tile_pool(name="ph3", bufs=1) as p3, \
             tc.tile_pool(name="psB", bufs=3, space="PSUM") as psB, \
             tc.tile_pool(name="psC", bufs=1, space="PSUM") as psC:

            # ---------------- constants ----------------
            ones_f = cp.tile([128, 128], F32, tag="ones_f")
            nc.gpsimd.memset(ones_f[:, :], 1.0)
            onesb = cp.tile([128, 128], BF16, tag="onesb")
            nc.scalar.copy(onesb[:, :], ones_f[:, :])
            # tri01[p, j] = 1 if j >= p else 0 (keep keys <= query), two
            # adjacent copies so one 3D-AP multiply masks both heads.
            tri01_f = cp.tile([128, 128], F32, tag="tri01_f")
            nc.gpsimd.affine_select(tri01_f[:, :], ones_f[:, :],
                                    pattern=[[1, 128]],
                                    compare_op=Alu.is_ge, fill=0.0,
                                    base=0, channel_multiplier=-1)
            tri2 = cp.tile([128, 256], BF16, tag="tri2")
            nc.scalar.copy(tri2[:, 0:128], tri01_f[:, :])
            nc.scalar.copy(tri2[:, 128:256], tri01_f[:, :])
            qb_s = cp.tile([128, 4], F32, tag="qb")
            nc.sync.dma_start(qb_s[:, :], qb_d[:, :])
            kb_s = cp.tile([128, 4], F32, tag="kb")
            nc.sync.dma_start(kb_s[:, :], kb_d[:, :])
            vb_s = cp.tile([1, OG], BF16, tag="vb")
            nc.sync.dma_start(vb_s[:, :], vb_d[:, :])
            kmask_s = cp.tile([128, SC], F32, tag="km")
            nc.sync.dma_start(kmask_s[:, :], kmask_d[:, :])

            def load_x_quarter(q):
                qs = q * W
                xT_t = []
                for kc in range(KC):
                    xt = xp.tile([128, W], BF16, tag=f"xT{kc}",
                                 name=f"xT{q}_{kc}")
                    nc.sync.dma_start(
                        xt[:, :], xT_d[kc * 128:(kc + 1) * 128, qs:qs + W])
                    xT_t.append(xt)
                return xT_t

            # quarter-0 activations first so the first projection matmuls
            # can start while the bulk of the weights still stream in
            x0_t = load_x_quarter(0)

            # ---------------- weights (loaded once) ----------------
            wq_t, wk_t, wv_t = [], [], []
            for kc in range(KC):
                for lst, pool, src in ((wq_t, wqp, wqT_d), (wk_t, wkp, wkT_d),
                                       (wv_t, wvp, wvT_d)):
                    wt = pool.tile([128, OG], BF16, tag=f"w{kc}",
                                   name=f"w{len(lst)}_{kc}")
                    nc.sync.dma_start(wt[:, :],
                                      src[kc * 128:(kc + 1) * 128, :])
                    lst.append(wt)
            wo_t = []
            for kc in range(4):
                wo = wop.tile([128, HID], BF16, tag=f"wo{kc}")
                nc.sync.dma_start(wo[:, :], woT_d[kc * 128:(kc + 1) * 128, :])
                wo_t.append(wo)

            # ---------------- persistent activation tiles ----------------
            qT_t = [qTp.tile([128, SEQ], BF16, tag=f"qT{i}", name=f"qT{i}")
                    for i in range(NPAIR)]
            kT_t = [kTp.tile([128, SEQ], BF16, tag=f"kT{i}", name=f"kT{i}")
                    for i in range(NPAIR)]
            v_t = [vp.tile([128, HPG * 65], BF16, tag=f"v{i}", name=f"v{i}")
                   for i in range(SC)]
            for i in range(SC):
                vv = v_t[i].rearrange("p (h c) -> p h c", c=65)
                nc.gpsimd.memset(vv[:, :, 64:65], 1.0)
            attnT_t = [aTp.tile([128, SEQ], BF16, tag=f"aT{i}", name=f"aT{i}")
                       for i in range(NPAIR)]

            def phase1_quarter(q, xT_t=None):
                qs = q * W
                if xT_t is None:
                    xT_t = load_x_quarter(q)
                # Q/K projections: out partitions = proj dims, cols = seq
                for w_t, o_t, bias in ((wq_t, qT_t, qb_s), (wk_t, kT_t, kb_s)):
                    for oc in range(4):
                        pqk = psC.tile([128, W], F32, tag="c512", name="pqk")
                        for kc in range(KC):
                            nc.tensor.matmul(
                                pqk[:, :],
                                w_t[kc][:, oc * 128:(oc + 1) * 128],
                                xT_t[kc][:, :],
                                start=(kc == 0), stop=(kc == KC - 1))
                        nc.vector.tensor_scalar_add(
                            o_t[oc][:, qs:qs + W], pqk[:, :],
                            bias[:, oc:oc + 1])
                # V projection: out partitions = seq chunk, cols = proj dims
                for sc in range(4):
                    scg = 4 * q + sc
                    pv = psC.tile([128, OG], F32, tag="c512", name="pv")
                    for kc in range(KC):
                        nc.tensor.matmul(
                            pv[:, :],
                            xT_t[kc][:, sc * 128:(sc + 1) * 128],
                            wv_t[kc][:, :],
                            start=(kc == 0), stop=False)
                    nc.tensor.matmul(pv[:, :], onesb[0:1, :], vb_s[0:1, :],
                                     start=False, stop=True)
                    src = pv.rearrange("p (h c) -> p h c", c=64)
                    dst = v_t[scg].rearrange("p (h c) -> p h c", c=65)
                    nc.vector.tensor_copy(dst[:, :, 0:64], src[:, :, :])

            def phase2_window(w, psA):
                ws = w * W
                chunks = [(c, 0) for c in range(4 * w)]
                chunks += [(4 * w + i, 128 * i) for i in range(4)]
                last = len(chunks) - 1
                for pr in range(NPAIR):
                    he = 2 * pr       # even head (rows 0:64)
                    at_e = psB.tile([128, W], F32, tag="b512", name="at_e")
                    at_o = psB.tile([128, W], F32, tag="b512", name="at_o")
                    for idx, (c, off) in enumerate(chunks):
                        n = W - off
                        sp = psA.tile([128, 2 * W], F32, tag="sp", name="sp")
                        sp3 = sp.rearrange("p (g c) -> p g c", g=2)
                        nc.tensor.matmul(
                            sp[:, off:W],
                            kT_t[pr][0:64, c * 128:(c + 1) * 128],
                            qT_t[pr][0:64, ws + off:ws + W],
                            start=True, stop=True)
                        nc.tensor.matmul(
                            sp[:, W + off:2 * W],
                            kT_t[pr][64:128, c * 128:(c + 1) * 128],
                            qT_t[pr][64:128, ws + off:ws + W],
                            start=True, stop=True)
                        et = p2.tile([128, 2 * W], BF16, tag="E", bufs=6)
                        et3 = et.rearrange("p (g c) -> p g c", g=2)
                        nc.scalar.activation(et3[:, :, off:W],
                                             sp3[:, :, off:W], AF.Exp,
                                             bias=kmask_s[:, c:c + 1],
                                             scale=SCALE)
                        if off or c == 4 * w:  # diagonal chunk
                            nc.vector.tensor_mul(
                                et3[:, :, off:off + 128],
                                et3[:, :, off:off + 128],
                                tri2.rearrange("p (g c) -> p g c", g=2))
                        nc.tensor.matmul(
                            at_e[0:65, off:W],
                            v_t[c][:, he * 65:(he + 1) * 65],
                            et[:, off:W],
                            start=(idx == 0), stop=(idx == last))
                        nc.tensor.matmul(
                            at_o[0:65, off:W],
                            v_t[c][:, (he + 1) * 65:(he + 2) * 65],
                            et[:, W + off:2 * W],
                            start=(idx == 0), stop=(idx == last))
                    # softmax division, off the PE critical path:
                    # evict unnormalized att + denominator row quickly
                    # (frees the PSUM slot), reciprocal on idle GpSimd,
                    # broadcast across partitions via a K=1 matmul.
                    for h, at in ((he, at_e), (he + 1, at_o)):
                        attnU = p2.tile([64, W], BF16, tag="aU", bufs=3)
                        nc.vector.tensor_copy(attnU[:, :], at[0:64, :])
                        dnr = p2.tile([128, W], F32, tag="dnr", bufs=2)
                        nc.vector.tensor_copy(dnr[64:65, :], at[64:65, :])
                        # reciprocal: DVE divide costs 8 cyc per FREE elem,
                        # so reshape the row to [128, 4] via DMA first
                        dnT = p2.tile([128, 4], F32, tag="dnT", bufs=2)
                        nc.sync.dma_start(dnT[:, :], dnr[64:65, :])
                        dnTr = p2.tile([128, 4], BF16, tag="dnTr", bufs=2)
                        with nc.allow_low_precision("recip"):
                            nc.vector.reciprocal(dnTr[:, :], dnT[:, :])
                        rcp = p2.tile([128, W], BF16, tag="rcp", bufs=2)
                        nc.sync.dma_start(rcp[64:65, :], dnTr[:, :])
                        # broadcast the reciprocal row back into the (now
                        # dead) at tile -- reuses its PSUM bank, WAW-ordered
                        # behind the two evictions above
                        nc.tensor.matmul(at[0:64, :], onesb[64:65, 0:64],
                                         rcp[64:65, :], start=True, stop=True)
                        if h % 2 == 0:
                            nc.vector.tensor_mul(
                                attnT_t[pr][0:64, ws:ws + W],
                                attnU[:, :], at[0:64, :])
                        else:
                            tmp = p2.tile([64, W], BF16, tag="tm", bufs=2)
                            nc.vector.tensor_mul(tmp[:, :], attnU[:, :],
                                                 at[0:64, :])
                            nc.sync.dma_start(attnT_t[pr][64:128, ws:ws + W],
                                              tmp[:, :])

            def phase3_window(w, pool, tag):
                # output projection for the sq chunks of window w
                for sc in range(4 * w, 4 * w + 4):
                    ot = p3.tile([128, HID], F32, tag="ou", bufs=3)
                    for n in range(2):
                        po = pool.tile([128, W], F32, tag=tag, name="po")
                        for kc in range(4):
                            nc.tensor.matmul(
                                po[:, :],
                                attnT_t[kc][:, sc * 128:(sc + 1) * 128],
                                wo_t[kc][:, n * W:(n + 1) * W],
                                start=(kc == 0), stop=(kc == 3))
                        nc.vector.tensor_copy(ot[:, n * W:(n + 1) * W],
                                              po[:, :])
                    nc.sync.dma_start(out_d[sc * 128:(sc + 1) * 128, :],
                                      ot[:, :])

            # interleave projection quarters, attention windows, and output
            # projection so the scheduler can fill PE idle time during
            # ScalarE-bound (softmax) stretches; the last window's output
            # projection runs after the scores pool closes, in a wider pool
            with tc.tile_pool(name="psA", bufs=2, space="PSUM") as psA:
                phase1_quarter(0, x0_t)
                phase2_window(0, psA)
                phase1_quarter(1)
                phase2_window(1, psA)
                phase1_quarter(2)
                phase2_window(2, psA)
                phase3_window(0, psC, "c512")
                phase1_quarter(3)
                phase2_window(3, psA)
                phase3_window(1, psC, "c512")
                phase3_window(2, psC, "c512")
            with tc.tile_pool(name="psD", bufs=4, space="PSUM") as psD:
                phase3_window(3, psD, "d512")

    nc.compile()
    return nc


def kernel(hidden_states, causal_mask, padding_mask,
           q_w, q_b, k_w, k_b, v_w, v_b, o_w, o_b):
    global _compiled
    from concourse.bass_utils import run_bass_kernel_spmd
    import ml_dtypes

    BF = ml_dtypes.bfloat16

    hidden_states = np.asarray(hidden_states, dtype=np.float32)
    padding_mask = np.asarray(padding_mask)
    q_w = np.asarray(q_w, dtype=np.float32)
    k_w = np.asarray(k_w, dtype=np.float32)
    v_w = np.asarray(v_w, dtype=np.float32)
    o_w = np.asarray(o_w, dtype=np.float32)
    q_b = np.asarray(q_b, dtype=np.float32)
    k_b = np.asarray(k_b, dtype=np.float32)
    v_b = np.asarray(v_b, dtype=np.float32)
    o_b = np.asarray(o_b, dtype=np.float32)

    if _compiled is None:
        _compiled = _build()
    nc = _compiled

    in_maps = []
    for b in range(BS):
        xT = np.ascontiguousarray(hidden_states[b].T).astype(BF)
        kmask = np.where(padding_mask[b], np.float32(-30000.0),
                         np.float32(0.0)).astype(np.float32)
        kmask2 = np.ascontiguousarray(kmask.reshape(SC, 128).T)
        for g in range(HG):
            r = slice(g * OG, (g + 1) * OG)
            in_maps.append({
                "xT": xT,
                "wqT": np.ascontiguousarray(q_w[r].T).astype(BF),
                "wkT": np.ascontiguousarray(k_w[r].T).astype(BF),
                "wvT": np.ascontiguousarray(v_w[r].T).astype(BF),
                "woT": np.ascontiguousarray(o_w[:, r].T).astype(BF),
                "qb": np.ascontiguousarray(q_b[r].reshape(4, 128).T),
                "kb": np.ascontiguousarray(k_b[r].reshape(4, 128).T),
                "vb": np.ascontiguousarray(v_b[r].reshape(1, OG)).astype(BF),
                "kmask": kmask2,
            })

    trace = os.environ.get("KERNEL_TRACE") == "1"
    res = run_bass_kernel_spmd(nc, in_maps, core_ids=list(range(NCORES)),
                               trace=trace)
    if trace and res.exec_time_ns is not None:
        print(f"HW exec time: {res.exec_time_ns} ns")
        if res.instructions_and_trace:
            print(f"trace: {res.instructions_and_trace[1]}")

    out = np.empty((BS, SEQ, HID), dtype=np.float32)
    for b in range(BS):
        out[b] = (res.results[2 * b]["out"] + res.results[2 * b + 1]["out"]
                  + o_b[None, :])
    return out


# revision 25
# speedup vs baseline: 1.8570x; 1.0375x over previous
"""Multi-head attention (bs=4, seq=2048, hidden=1024, 16 heads) on 8 trn2 cores.

Sharding: core = (batch b, head-group g) with 4 batches x 2 groups of 8 heads.
Each core computes QKV projections for its head slice, causal+padded softmax
attention, and a partial output projection; the host sums the two partial
outputs per batch and adds o_b.

v2 layout notes:
  - bf16 weights/activations in SBUF (fp32 accumulate in PSUM); host converts.
  - phase-1 projections run in four 512-seq quarters, emitted interleaved
    with the four 512-query attention windows so the Tile scheduler can keep
    the PE busy with projection matmuls while ScalarE runs softmax exps.
  - score matmuls for a head pair issue to disjoint 64-row PE groups
    (base partitions 0 and 64) so they execute concurrently.
  - exp runs once per (chunk, head-pair) as a single wide ACT op over a
    [128, 2, w] access pattern; padding mask rides as a per-partition bias.
  - softmax division: denominator rows leave PSUM via ScalarE ln, the
    reciprocal is exp(-ln) (same ACT table set), GpSimd broadcasts it
    across partitions, DVE does the final multiply.
"""
import os
import sys

for _p in ("/opt/trn_rl_repo",):
    if _p not in sys.path:
        sys.path.insert(0, _p)

import numpy as np

HID = 1024
HEADS = 16
D = 64
BS = 4
SEQ = 2048
NCORES = 8
HG = 2             # head groups (tensor-parallel axis)
HPG = HEADS // HG  # 8 heads per core
NPAIR = HPG // 2   # 4 head pairs per core
OG = HPG * D       # 512 projection dims per core
KC = HID // 128    # 8 hidden chunks
W = 512            # query window
NW = SEQ // W      # 4 windows (== phase-1 quarters)
SC = SEQ // 128    # 16 key chunks
SCALE = 1.0 / np.sqrt(D)

_compiled = None


def _build():
    import concourse.tile as tile
    from concourse import bacc, mybir

    F32 = mybir.dt.float32
    BF16 = mybir.dt.bfloat16
    AF = mybir.ActivationFunctionType
    Alu = mybir.AluOpType

    nc = bacc.Bacc("TRN2", target_bir_lowering=False, debug=False,
                   num_devices=NCORES)

    xT_d = nc.dram_tensor("xT", [HID, SEQ], BF16, kind="ExternalInput").ap()
    wqT_d = nc.dram_tensor("wqT", [HID, OG], BF16, kind="ExternalInput").ap()
    wkT_d = nc.dram_tensor("wkT", [HID, OG], BF16, kind="ExternalInput").ap()
    wvT_d = nc.dram_tensor("wvT", [HID, OG], BF16, kind="ExternalInput").ap()
    woT_d = nc.dram_tensor("woT", [OG, HID], BF16, kind="ExternalInput").ap()
    qb_d = nc.dram_tensor("qb", [128, 4], F32, kind="ExternalInput").ap()
    kb_d = nc.dram_tensor("kb", [128, 4], F32, kind="ExternalInput").ap()
    vb_d = nc.dram_tensor("vb", [1, OG], BF16, kind="ExternalInput").ap()
    kmask_d = nc.dram_tensor("kmask", [128, SC], F32, kind="ExternalInput").ap()
    out_d = nc.dram_tensor("out", [SEQ, HID], BF16,
                           kind="ExternalOutput").ap()

    with tile.TileContext(nc) as tc:
        with tc.tile_pool(name="const", bufs=1) as cp, \
             tc.tile_pool(name="wq", bufs=1) as wqp, \
             tc.tile_pool(name="wk", bufs=1) as wkp, \
             tc.tile_pool(name="wv", bufs=1) as wvp, \
             tc.tile_pool(name="wo", bufs=1) as wop, \
             tc.tile_pool(name="qT", bufs=1) as qTp, \
             tc.tile_pool(name="kT", bufs=1) as kTp, \
             tc.tile_pool(name="v", bufs=1) as vp, \
             tc.tile_pool(name="attnT", bufs=1) as aTp, \
             tc.tile_pool(name="x", bufs=2) as xp, \
             tc.tile_pool(name="ph2", bufs=1) as p2, \
             tc.tile_pool(name="ph3", bufs=1) as p3, \
             tc.tile_pool(name="psB", bufs=3, space="PSUM") as psB, \
             tc.tile_pool(name="psC", bufs=1, space="PSUM") as psC:

            # ---------------- constants ----------------
            ones_f = cp.tile([128, 128], F32, tag="ones_f")
            nc.gpsimd.memset(ones_f[:, :], 1.0)
            onesb = cp.tile([128, 128], BF16, tag="onesb")
            nc.scalar.copy(onesb[:, :], ones_f[:, :])
            # tri01[p, j] = 1 if j >= p else 0 (keep keys <= query), two
            # adjacent copies so one 3D-AP multiply masks both heads.
            tri01_f = cp.tile([128, 128], F32, tag="tri01_f")
            nc.gpsimd.affine_select(tri01_f[:, :], ones_f[:, :],
                                    pattern=[[1, 128]],
                                    compare_op=Alu.is_ge, fill=0.0,
                                    base=0, channel_multiplier=-1)
            tri2 = cp.tile([128, 256], BF16, tag="tri2")
            nc.scalar.copy(tri2[:, 0:128], tri01_f[:, :])
            nc.scalar.copy(tri2[:, 128:256], tri01_f[:, :])
            qb_s = cp.tile([128, 4], F32, tag="qb")
            nc.sync.dma_start(qb_s[:, :], qb_d[:, :])
            kb_s = cp.tile([128, 4], F32, tag="kb")
            nc.sync.dma_start(kb_s[:, :], kb_d[:, :])
            vb_s = cp.tile([1, OG], BF16, tag="vb")
            nc.sync.dma_start(vb_s[:, :], vb_d[:, :])
            kmask_s = cp.tile([128, SC], F32, tag="km")
            nc.sync.dma_start(kmask_s[:, :], kmask_d[:, :])

            def load_x_quarter(q):
                qs = q * W
                xT_t = []
                for kc in range(KC):
                    xt = xp.tile([128, W], BF16, tag=f"xT{kc}",
                                 name=f"xT{q}_{kc}")
                    nc.sync.dma_start(
                        xt[:, :], xT_d[kc * 128:(kc + 1) * 128, qs:qs + W])
                    xT_t.append(xt)
                return xT_t

            # quarter-0 activations first so the first projection matmuls
            # can start while the bulk of the weights still stream in
            x0_t = load_x_quarter(0)

            # ---------------- weights (loaded once) ----------------
            # q/k first (quarter-0 needs them before v), v next, wo last
            wq_t, wk_t, wv_t = [], [], []
            for kc in range(KC):
                for lst, pool, src in ((wq_t, wqp, wqT_d),
                                       (wk_t, wkp, wkT_d)):
                    wt = pool.tile([128, OG], BF16, tag=f"w{kc}",
                                   name=f"w{len(lst)}_{kc}")
                    nc.sync.dma_start(wt[:, :],
                                      src[kc * 128:(kc + 1) * 128, :])
                    lst.append(wt)
            for kc in range(KC):
                wt = wvp.tile([128, OG], BF16, tag=f"w{kc}", name=f"wv_{kc}")
                nc.sync.dma_start(wt[:, :],
                                  wvT_d[kc * 128:(kc + 1) * 128, :])
                wv_t.append(wt)
            wo_t = []
            for kc in range(4):
                wo = wop.tile([128, HID], BF16, tag=f"wo{kc}")
                nc.sync.dma_start(wo[:, :], woT_d[kc * 128:(kc + 1) * 128, :])
                wo_t.append(wo)

            # ---------------- persistent activation tiles ----------------
            qT_t = [qTp.tile([128, SEQ], BF16, tag=f"qT{i}", name=f"qT{i}")
                    for i in range(NPAIR)]
            kT_t = [kTp.tile([128, SEQ], BF16, tag=f"kT{i}", name=f"kT{i}")
                    for i in range(NPAIR)]
            v_t = [vp.tile([128, HPG * 65], BF16, tag=f"v{i}", name=f"v{i}")
                   for i in range(SC)]
            for i in range(SC):
                vv = v_t[i].rearrange("p (h c) -> p h c", c=65)
                nc.gpsimd.memset(vv[:, :, 64:65], 1.0)
            attnT_t = [aTp.tile([128, SEQ], BF16, tag=f"aT{i}", name=f"aT{i}")
                       for i in range(NPAIR)]

            def phase1_quarter(q, xT_t=None):
                qs = q * W
                if xT_t is None:
                    xT_t = load_x_quarter(q)
                # Q/K projections: out partitions = proj dims, cols = seq
                for w_t, o_t, bias in ((wq_t, qT_t, qb_s), (wk_t, kT_t, kb_s)):
                    for oc in range(4):
                        pqk = psC.tile([128, W], F32, tag="c512", name="pqk")
                        for kc in range(KC):
                            nc.tensor.matmul(
                                pqk[:, :],
                                w_t[kc][:, oc * 128:(oc + 1) * 128],
                                xT_t[kc][:, :],
                                start=(kc == 0), stop=(kc == KC - 1))
                        nc.vector.tensor_scalar_add(
                            o_t[oc][:, qs:qs + W], pqk[:, :],
                            bias[:, oc:oc + 1])
                # V projection: out partitions = seq chunk, cols = proj dims
                for sc in range(4):
                    scg = 4 * q + sc
                    pv = psC.tile([128, OG], F32, tag="c512", name="pv")
                    for kc in range(KC):
                        nc.tensor.matmul(
                            pv[:, :],
                            xT_t[kc][:, sc * 128:(sc + 1) * 128],
                            wv_t[kc][:, :],
                            start=(kc == 0), stop=False)
                    nc.tensor.matmul(pv[:, :], onesb[0:1, :], vb_s[0:1, :],
                                     start=False, stop=True)
                    src = pv.rearrange("p (h c) -> p h c", c=64)
                    dst = v_t[scg].rearrange("p (h c) -> p h c", c=65)
                    nc.vector.tensor_copy(dst[:, :, 0:64], src[:, :, :])

            def phase2_window(w, psA):
                ws = w * W
                chunks = [(c, 0) for c in range(4 * w)]
                chunks += [(4 * w + i, 128 * i) for i in range(4)]
                last = len(chunks) - 1
                for pr in range(NPAIR):
                    he = 2 * pr       # even head (rows 0:64)
                    at_e = psB.tile([128, W], F32, tag="b512", name="at_e")
                    at_o = psB.tile([128, W], F32, tag="b512", name="at_o")
                    for idx, (c, off) in enumerate(chunks):
                        n = W - off
                        sp = psA.tile([128, 2 * W], F32, tag="sp", name="sp")
                        sp3 = sp.rearrange("p (g c) -> p g c", g=2)
                        nc.tensor.matmul(
                            sp[:, off:W],
                            kT_t[pr][0:64, c * 128:(c + 1) * 128],
                            qT_t[pr][0:64, ws + off:ws + W],
                            start=True, stop=True)
                        nc.tensor.matmul(
                            sp[:, W + off:2 * W],
                            kT_t[pr][64:128, c * 128:(c + 1) * 128],
                            qT_t[pr][64:128, ws + off:ws + W],
                            start=True, stop=True)
                        et = p2.tile([128, 2 * W], BF16, tag="E", bufs=6)
                        et3 = et.rearrange("p (g c) -> p g c", g=2)
                        nc.scalar.activation(et3[:, :, off:W],
                                             sp3[:, :, off:W], AF.Exp,
                                             bias=kmask_s[:, c:c + 1],
                                             scale=SCALE)
                        if off or c == 4 * w:  # diagonal chunk
                            nc.vector.tensor_mul(
                                et3[:, :, off:off + 128],
                                et3[:, :, off:off + 128],
                                tri2.rearrange("p (g c) -> p g c", g=2))
                        nc.tensor.matmul(
                            at_e[0:65, off:W],
                            v_t[c][:, he * 65:(he + 1) * 65],
                            et[:, off:W],
                            start=(idx == 0), stop=(idx == last))
                        nc.tensor.matmul(
                            at_o[0:65, off:W],
                            v_t[c][:, (he + 1) * 65:(he + 2) * 65],
                            et[:, W + off:2 * W],
                            start=(idx == 0), stop=(idx == last))
                    # softmax division, off the PE critical path:
                    # evict unnormalized att + denominator row quickly
                    # (frees the PSUM slot), reciprocal on idle GpSimd,
                    # broadcast across partitions via a K=1 matmul.
                    for h, at in ((he, at_e), (he + 1, at_o)):
                        attnU = p2.tile([64, W], BF16, tag="aU", bufs=3)
                        nc.vector.tensor_copy(attnU[:, :], at[0:64, :])
                        dnr = p2.tile([128, W], F32, tag="dnr", bufs=2)
                        nc.vector.tensor_copy(dnr[64:65, :], at[64:65, :])
                        # reciprocal: DVE divide costs 8 cyc per FREE elem,
                        # so reshape the row to [128, 4] via DMA first
                        dnT = p2.tile([128, 4], F32, tag="dnT", bufs=2)
                        nc.sync.dma_start(dnT[:, :], dnr[64:65, :])
                        dnTr = p2.tile([128, 4], BF16, tag="dnTr", bufs=2)
                        with nc.allow_low_precision("recip"):
                            nc.vector.reciprocal(dnTr[:, :], dnT[:, :])
                        rcp = p2.tile([128, W], BF16, tag="rcp", bufs=2)
                        nc.sync.dma_start(rcp[64:65, :], dnTr[:, :])
                        # broadcast the reciprocal row back into the (now
                        # dead) at tile -- reuses its PSUM bank, WAW-ordered
                        # behind the two evictions above
                        nc.tensor.matmul(at[0:64, :], onesb[64:65, 0:64],
                                         rcp[64:65, :], start=True, stop=True)
                        if h % 2 == 0:
                            nc.vector.tensor_mul(
                                attnT_t[pr][0:64, ws:ws + W],
                                attnU[:, :], at[0:64, :])
                        else:
                            tmp = p2.tile([64, W], BF16, tag="tm", bufs=2)
                            nc.vector.tensor_mul(tmp[:, :], attnU[:, :],
                                                 at[0:64, :])
                            nc.sync.dma_start(attnT_t[pr][64:128, ws:ws + W],
                                              tmp[:, :])

            def phase3_window(w, pool, tag):
                # output projection for the sq chunks of window w
                for sc in range(4 * w, 4 * w + 4):
                    ot = p3.tile([128, HID], BF16, tag="ou", bufs=3)
                    for n in range(2):
                        po = pool.tile([128, W], F32, tag=tag, name="po")
                        for kc in range(4):
                            nc.tensor.matmul(
                                po[:, :],
                                attnT_t[kc][:, sc * 128:(sc + 1) * 128],
                                wo_t[kc][:, n * W:(n + 1) * W],
                                start=(kc == 0), stop=(kc == 3))
                        nc.vector.tensor_copy(ot[:, n * W:(n + 1) * W],
                                              po[:, :])
                    nc.sync.dma_start(out_d[sc * 128:(sc + 1) * 128, :],
                                      ot[:, :])

            # interleave projection quarters, attention windows, and output
            # projection so the scheduler can fill PE idle time during
            # ScalarE-bound (softmax) stretches; the last window's output
            # projection runs after the scores pool closes, in a wider pool
            with tc.tile_pool(name="psA", bufs=2, space="PSUM") as psA:
                phase1_quarter(0, x0_t)
                phase2_window(0, psA)
                phase1_quarter(1)
                phase2_window(1, psA)
                phase1_quarter(2)
                phase2_window(2, psA)
                phase3_window(0, psC, "c512")
                phase1_quarter(3)
                phase2_window(3, psA)
                phase3_window(1, psC, "c512")
                phase3_window(2, psC, "c512")
            with tc.tile_pool(name="psD", bufs=4, space="PSUM") as psD:
                phase3_window(3, psD, "d512")

    nc.compile()
    return nc


def kernel(hidden_states, causal_mask, padding_mask,
           q_w, q_b, k_w, k_b, v_w, v_b, o_w, o_b):
    global _compiled
    from concourse.bass_utils import run_bass_kernel_spmd
    import ml_dtypes

    BF = ml_dtypes.bfloat16

    hidden_states = np.asarray(hidden_states, dtype=np.float32)
    padding_mask = np.asarray(padding_mask)
    q_w = np.asarray(q_w, dtype=np.float32)
    k_w = np.asarray(k_w, dtype=np.float32)
    v_w = np.asarray(v_w, dtype=np.float32)
    o_w = np.asarray(o_w, dtype=np.float32)
    q_b = np.asarray(q_b, dtype=np.float32)
    k_b = np.asarray(k_b, dtype=np.float32)
    v_b = np.asarray(v_b, dtype=np.float32)
    o_b = np.asarray(o_b, dtype=np.float32)

    if _compiled is None:
        _compiled = _build()
    nc = _compiled

    in_maps = []
    for b in range(BS):
        xT = np.ascontiguousarray(hidden_states[b].T).astype(BF)
        kmask = np.where(padding_mask[b], np.float32(-30000.0),
                         np.float32(0.0)).astype(np.float32)
        kmask2 = np.ascontiguousarray(kmask.reshape(SC, 128).T)
        for g in range(HG):
            r = slice(g * OG, (g + 1) * OG)
            in_maps.append({
                "xT": xT,
                "wqT": np.ascontiguousarray(q_w[r].T).astype(BF),
                "wkT": np.ascontiguousarray(k_w[r].T).astype(BF),
                "wvT": np.ascontiguousarray(v_w[r].T).astype(BF),
                "woT": np.ascontiguousarray(o_w[:, r].T).astype(BF),
                "qb": np.ascontiguousarray(q_b[r].reshape(4, 128).T),
                "kb": np.ascontiguousarray(k_b[r].reshape(4, 128).T),
                "vb": np.ascontiguousarray(v_b[r].reshape(1, OG)).astype(BF),
                "kmask": kmask2,
            })

    trace = os.environ.get("KERNEL_TRACE") == "1"
    res = run_bass_kernel_spmd(nc, in_maps, core_ids=list(range(NCORES)),
                               trace=trace)
    if trace and res.exec_time_ns is not None:
        print(f"HW exec time: {res.exec_time_ns} ns")
        if res.instructions_and_trace:
            print(f"trace: {res.instructions_and_trace[1]}")

    out = np.empty((BS, SEQ, HID), dtype=np.float32)
    for b in range(BS):
        out[b] = (res.results[2 * b]["out"].astype(np.float32)
                  + res.results[2 * b + 1]["out"].astype(np.float32)
                  + o_b[None, :])
    return out
